# revision 56
# baseline (speedup 1.0000x reference)
"""Trainium2 Bass kernel for a 3-layer GCN + MLP scorer with neighbor-masked softmax.

The reference computes, for a graph with N nodes / E edges:
    h = tanh(GCN(tanh(GCN(tanh(GCN(x)))))); scores = MLP(h)
    out = softmax(scores masked to out-neighbors of current_vertex_idx)

The softmax mask makes the output exactly zero outside M = {out-neighbors of
cvi} | {cvi}.  Only the 3-hop *in*-neighborhood of M (a few hundred nodes of
the 50k) can influence the masked scores, so the kernel prunes the graph to
that closure on the host, builds small dense aggregation matrices (adjacency
with GCN normalization baked in), and runs the entire floating-point
computation on-device as a chain of dense matmuls + activations.  The device
program is SPMD-replicated across the 8 NeuronCores.

Host work is index-only (degree counts, BFS closure, packing the per-call
aggregation matrices); every FLOP of the model runs on the NeuronCores.

Device-side notes (v2):
  - All matmul operands are bf16 (fp32 PSUM accumulate): single-pass matmuls
    (fp32 needs 2 half-speed passes) and half the HBM traffic.  The softmax
    tail stays fp32.
  - Weights stream over both hardware DGE queues (sync + scalar), ordered by
    first use (W2 halves first, Wp1 last); W2/W3 are split into column halves
    so the first half of t = h @ W overlaps the second half's DMA.
  - Layer 1 is reassociated as (A1 @ x0) @ W1 (contract the node dim first at
    F_IN=16 wide); layouts alternate node-major/feature-major so every matmul
    contracts on partitions with no transposes.
  - Softmax is restricted to the first len(M) lanes (padding lanes are never
    read), which removes the -inf mask row; bp2 is dropped (softmax is
    invariant to constant shifts).  Leaky-relu is a single Prelu activation
    with the bias folded in via a K=1 matmul.
"""

import numpy as np
import ml_dtypes

D = 512      # node embedding size
H = 256      # predictor hidden size
F_IN = 16    # raw node feature dim
ALPHA = 0.1  # leaky relu slope
N_CORES = 8
KD = D // 128
KH = H // 128
N_WARMUP = 4  # dummy matmuls that ramp the PE clock during the DMA window

bf16 = ml_dtypes.bfloat16
fp8 = ml_dtypes.float8_e4m3

# Bucket caps: beyond these we fall back to the (identical-math) numpy path.
# n1/n2 <= 64 so the aggregation operands live in partition rows 0-63 (the
# "lo" half of the split input blob).
MAX_N0 = 4096
MAX_N1 = 64
MAX_N2 = 64
MAX_N3 = 64

_prog_cache: dict[tuple, object] = {}
last_results = None  # BassKernelResults of the most recent device run


def _blob_layout(bucket):
    """Column layout of the input blob A (bf16).  Columns [0, _hi) are
    needed on all 128 partition rows and are DMA'd as two row-halves (the
    DGE is descriptor-rate-bound, one descriptor per row, so row-splitting
    across both queues halves the wall time).  Columns [_hi, _total) only
    ever feed partition rows < 64 (w1 uses 16 rows, a2/a3 use n1/n2 <= 64)
    and ride only in the lo-half parameter."""
    n0, n1, n2, n3 = bucket
    k0 = n0 // 128
    off = 0
    lay = {}
    lay["x0"] = off; off += k0 * F_IN
    lay["a1"] = off; off += k0 * n1
    lay["b1"] = off; off += KD
    lay["b2"] = off; off += KD
    lay["b3"] = off; off += KD
    lay["bp1"] = off; off += KH     # bp1 feature-major columns
    lay["wp2"] = off; off += KH     # feature-major columns
    lay["_hi"] = off                # hi-half parameter covers [0, _hi)
    lay["a2"] = off; off += n2      # rows 0..n1
    lay["a3"] = off; off += n3      # rows 0..n2
    lay["_total"] = off
    return lay


# --------------------------------------------------------------------------
# Device program
# --------------------------------------------------------------------------

def _build_program(key):
    import concourse.bass as bass
    import concourse.bacc as bacc
    import concourse.mybir as mybir
    import concourse.tile as tile

    bucket, lenM, zero_bias = key
    n0, n1, n2, n3 = bucket
    f32 = mybir.dt.float32
    b16 = mybir.dt.bfloat16
    f8 = mybir.dt.float8e4
    k0 = n0 // 128
    Tanh = mybir.ActivationFunctionType.Tanh
    Prelu = mybir.ActivationFunctionType.Prelu
    Exp = mybir.ActivationFunctionType.Exp
    Copy = mybir.ActivationFunctionType.Copy
    DR = mybir.MatmulPerfMode.DoubleRow
    lay = _blob_layout(bucket)
    CA = lay["_total"]
    CH = lay["_hi"]
    DH = D // 2   # 256: t1/t2 PSUM->SBUF cast half width

    nc = bacc.Bacc("TRN2", target_bir_lowering=False, debug=False)
    P_w1 = nc.declare_dram_parameter("w1", [F_IN, D], b16, isOutput=False)
    P_Alo = nc.declare_dram_parameter("Alo", [64, CA], b16, isOutput=False)
    P_Ahi = nc.declare_dram_parameter("Ahi", [64, CH], b16, isOutput=False)
    P_w23 = nc.declare_dram_parameter("w23", [128, 2, KD, D], f8,
                                      isOutput=False)
    P_wp1 = nc.declare_dram_parameter("wp1", [128, KH * 2, 2, 128], f8,
                                      isOutput=False)
    P_out = nc.declare_dram_parameter("out", [1, lenM], f32, isOutput=True)

    with tile.TileContext(nc) as tc:
        with (
            tc.tile_pool(name="sb", bufs=1) as sb,
            tc.tile_pool(name="ps", bufs=4, space="PSUM") as ps,
            tc.tile_pool(name="pss", bufs=2, space="PSUM") as pss,
            tc.tile_pool(name="wu", bufs=1, space="PSUM") as wu,
        ):
            # ---- input DMAs: the DGE is descriptor-rate-bound (one
            # descriptor per partition row), so the blob A (whose delivery
            # gates the whole H1 chain) is row-split across both queues;
            # W2|W3 (4KB descriptors) fill the sync queue's second slot and
            # wp1 the scalar queue's.
            w1 = sb.tile([F_IN, D], b16, tag="w1")
            nc.sync.dma_start(w1[:], P_w1[:])
            A = sb.tile([128, CA], b16, tag="A")
            nc.sync.dma_start(A[0:64, :], P_Alo[:])
            nc.scalar.dma_start(A[64:128, 0:CH], P_Ahi[:])
            w23 = sb.tile([128, 2, KD, D], f8, tag="w23")
            nc.sync.dma_start(w23[:], P_w23[:])
            wp1 = sb.tile([128, KH * 2, 2, 128], f8, tag="wp1")
            nc.scalar.dma_start(wp1[:], P_wp1[:])

            # ---- PE warm-up: the tensor engine's clock ramps with use (low ->
            # mid p-state); a fat dummy absorbs the ramp and short keep-alives
            # hold the clock until the first real matmul.
            wu_sb = sb.tile([128, 512], b16, tag="wu_sb")
            nc.vector.memset(wu_sb[:], 0)
            wu_ps = wu.tile([128, 512], f32, tag="wu_ps")
            for i in range(N_WARMUP):
                w_cols = 512 if i < 1 else 64
                nc.tensor.matmul(wu_ps[:, 0:w_cols], wu_sb[:, 0:128],
                                 wu_sb[:, 0:w_cols], start=True, stop=True)
            wu_out = sb.tile([1, 1], f32, tag="wu_out")
            nc.vector.tensor_copy(wu_out[:], wu_ps[0:1, 0:1])

            def acol(name, i=0, w=1, p0=0, p1=128):
                return A[p0:p1, lay[name] + i:lay[name] + i + w]

            def act_bias(out_ap, acc_ap, func, bname, di, **kw):
                if zero_bias:
                    nc.scalar.activation(out_ap, acc_ap, func, **kw)
                else:
                    nc.scalar.activation(out_ap, acc_ap, func,
                                         bias=acol(bname, di), **kw)

            # ---- BT = (A1 @ x0).T : [F_IN, n1]  (contract n0 nodes)
            acc = pss.tile([F_IN, n1], f32, tag="pss")
            for j in range(k0):
                nc.tensor.matmul(acc[:], acol("x0", j * F_IN, F_IN),
                                 acol("a1", j * n1, n1),
                                 start=(j == 0), stop=(j == k0 - 1))
            bt = sb.tile([F_IN, n1], b16, tag="bt")
            nc.vector.tensor_copy(bt[:], acc[:])

            # ---- H1[:, di] = tanh(W1[:, dslice].T @ BT + b1) : feature-major
            # fp8 so the dense layers can run DoubleRow (2 K-rows/cycle).
            # Tanhs run per d-tile PAIR (separate PSUM tiles per pair: a
            # group's start=True zeroes bank-granular regions, so mixing
            # groups and readers in one bank serializes them).
            H1 = sb.tile([128, KD, n1], f8, tag="H1")
            for pr in range(KD // 2):
                accp2 = ps.tile([128, 2, n1], f32, tag="ps")
                for g in range(2):
                    di = 2 * pr + g
                    nc.tensor.matmul(accp2[:, g, :],
                                     w1[:, di * 128:(di + 1) * 128],
                                     bt[:], start=True, stop=True)
                    if not zero_bias:
                        act_bias(H1[:, di, :], accp2[:, g, :], Tanh, "b1", di)
                if zero_bias:
                    nc.scalar.activation(H1[:, 2 * pr:2 * pr + 2, :],
                                         accp2[:], Tanh)

            def dense_layer(Hf, n_rows, wi, lname):
                # t = h @ W : node-major [n_rows, D] via fp8 DoubleRow pairs;
                # PSUM->SBUF bf16 cast halves on vector+scalar engines.
                t_sb = sb.tile([n_rows, D], b16, tag=lname)
                acc = ps.tile([n_rows, D], f32, tag="ps")
                for p in range(KD // 2):
                    nc.tensor.matmul(acc[:], Hf[:, 2 * p:2 * p + 2, :],
                                     w23[:, wi, 2 * p:2 * p + 2, :],
                                     start=(p == 0), stop=(p == KD // 2 - 1),
                                     perf_mode=DR)
                for q in range(4):
                    qs = slice(q * 128, (q + 1) * 128)
                    nc.vector.tensor_copy(t_sb[:, qs], acc[:, qs])
                return t_sb

            def agg_layer(t_in, n_in, aname, n_out, bname, lname):
                # Hf[:, di] = tanh((A @ t).T + b) : feature-major, fp8,
                # per-pair PSUM tiles + tanh so t(next) pair 0 starts early
                Hf = sb.tile([128, KD, n_out], f8, tag=lname)
                for pr in range(KD // 2):
                    accp2 = ps.tile([128, 2, n_out], f32, tag="ps")
                    for g in range(2):
                        di = 2 * pr + g
                        nc.tensor.matmul(accp2[:, g, :],
                                         t_in[:, di * 128:(di + 1) * 128],
                                         acol(aname, 0, n_out, 0, n_in),
                                         start=True, stop=True)
                        if not zero_bias:
                            act_bias(Hf[:, di, :], accp2[:, g, :], Tanh,
                                     bname, di)
                    if zero_bias:
                        nc.scalar.activation(Hf[:, 2 * pr:2 * pr + 2, :],
                                             accp2[:], Tanh)
                return Hf

            t1 = dense_layer(H1, n1, 0, "t1")
            H2 = agg_layer(t1, n1, "a2", n2, "b2", "H2")
            t2 = dense_layer(H2, n2, 1, "t2")
            H3 = agg_layer(t2, n2, "a3", n3, "b3", "H3")

            # ---- predictor hidden: P[:, hi] = prelu(Wp1.T @ h3.T + bp1)
            # (fp8 DoubleRow over d-tile pairs; bp1 rides the activation bias)
            Pf = sb.tile([128, KH * n3], b16, tag="Pf")
            for hi in range(KH):
                accp = ps.tile([128, n3], f32, tag="ps")
                for p in range(KD // 2):
                    nc.tensor.matmul(
                        accp[:], wp1[:, hi * 2 + p, :, :],
                        H3[:, 2 * p:2 * p + 2, :], start=(p == 0),
                        stop=(p == KD // 2 - 1), perf_mode=DR)
                act_bias(Pf[:, hi * n3:(hi + 1) * n3], accp[:],
                         Prelu, "bp1", hi, alpha=ALPHA)

            # ---- scores (first lenM lanes only; softmax is shift-invariant
            # so bp2 is dropped, and padding lanes are never read)
            acc = pss.tile([1, n3], f32, tag="pss")
            for hi in range(KH):
                nc.tensor.matmul(acc[:], acol("wp2", hi),
                                 Pf[:, hi * n3:(hi + 1) * n3],
                                 start=(hi == 0), stop=(hi == KH - 1))
            e = sb.tile([1, lenM], f32, tag="e")
            nc.scalar.activation(e[:], acc[0:1, 0:lenM], Exp)
            ssum = sb.tile([1, 1], f32, tag="ssum")
            nc.vector.tensor_reduce(ssum[:], e[:], mybir.AxisListType.X,
                                    mybir.AluOpType.add)
            rs = sb.tile([1, 1], f32, tag="rs")
            nc.vector.reciprocal(rs[:], ssum[:])
            o = sb.tile([1, lenM], f32, tag="o")
            nc.vector.tensor_scalar_mul(o[:], e[:], rs[:])
            nc.sync.dma_start(P_out[:], o[:])

    nc.compile()
    return nc


def _get_program(key):
    prog = _prog_cache.get(key)
    if prog is None:
        prog = _build_program(key)
        _prog_cache[key] = prog
    return prog


_runner_cache: dict[tuple, dict] = {}
_dev_weights: dict[str, tuple] = {}
# inputs that rarely change between calls: keep them resident on-device
_WEIGHT_PARAMS = ("w1", "w23", "wp1")


def _get_runner(key, nc):
    """Compile-once executor for the SPMD program (the per-call jit rebuild in
    run_bass_kernel_spmd re-traces and re-compiles; this caches the compiled
    shard_map callable per bucket)."""
    r = _runner_cache.get(key)
    if r is not None:
        return r
    import jax
    import numpy as np
    from jax.sharding import Mesh, PartitionSpec
    from jax.experimental.shard_map import shard_map
    from concourse import bass2jax
    import concourse.mybir as mybir

    bass2jax.install_neuronx_cc_hook()
    partition_name = (nc.partition_id_tensor.name
                      if nc.partition_id_tensor else None)
    in_names, out_names, out_avals = [], [], []
    for alloc in nc.m.functions[0].allocations:
        if not isinstance(alloc, mybir.MemoryLocationSet):
            continue
        name = alloc.memorylocations[0].name
        if alloc.kind == "ExternalInput":
            if name != partition_name:
                in_names.append(name)
        elif alloc.kind == "ExternalOutput":
            out_names.append(name)
            out_avals.append(jax.core.ShapedArray(
                tuple(alloc.tensor_shape), mybir.dt.np(alloc.dtype)))
    n_params = len(in_names)
    all_names = in_names + out_names
    if partition_name is not None:
        all_names = all_names + [partition_name]
    all_names = tuple(all_names)

    def _body(*args):
        operands = list(args)
        if partition_name is not None:
            operands.append(bass2jax.partition_id_tensor())
        outs = bass2jax._bass_exec_p.bind(
            *operands, out_avals=tuple(out_avals), in_names=all_names,
            out_names=tuple(out_names), lowering_input_output_aliases=(),
            sim_require_finite=True, sim_require_nnan=True, nc=nc)
        return tuple(outs)

    devices = jax.devices()[:N_CORES]
    mesh = Mesh(np.asarray(devices), ("core",))
    in_specs = (PartitionSpec("core"),) * (n_params + len(out_names))
    out_specs = (PartitionSpec("core"),) * len(out_names)
    donate = tuple(range(n_params, n_params + len(out_names)))
    fn = jax.jit(
        shard_map(_body, mesh=mesh, in_specs=in_specs, out_specs=out_specs,
                  check_rep=False),
        donate_argnums=donate, keep_unused=True)
    r = dict(fn=fn, in_names=in_names, out_names=out_names,
             out_avals=out_avals, mesh=mesh)
    _runner_cache[key] = r
    return r


def _run_fast(key, nc, in_map):
    """Execute via the cached runner; returns core-0 output dict."""
    import jax
    from jax.sharding import NamedSharding, PartitionSpec

    r = _get_runner(key, nc)
    sharding = NamedSharding(r["mesh"], PartitionSpec("core"))
    args = []
    for name in r["in_names"]:
        arr = np.ascontiguousarray(in_map[name])
        if name in _WEIGHT_PARAMS:
            cached = _dev_weights.get(name)
            if cached is not None and cached[0].shape == arr.shape and \
                    np.array_equal(cached[0], arr):
                args.append(cached[1])
                continue
            dev = jax.device_put(
                np.concatenate([arr] * N_CORES, axis=0), sharding)
            _dev_weights[name] = (arr.copy(), dev)
            args.append(dev)
        else:
            args.append(np.concatenate([arr] * N_CORES, axis=0))
    zeros = [np.zeros((N_CORES * a.shape[0], *a.shape[1:]), a.dtype)
             for a in r["out_avals"]]
    outs = r["fn"](*args, *zeros)
    return {
        name: np.asarray(outs[i]).reshape(N_CORES, *r["out_avals"][i].shape)[0]
        for i, name in enumerate(r["out_names"])
    }


# --------------------------------------------------------------------------
# Host-side graph pruning / packing
# --------------------------------------------------------------------------

def _next_size(n, minimum):
    r = minimum
    while r < n:
        r *= 2
    return r


def _prune(N, src, dst, cvi):
    """Return (M, levels, edges, norms) for the 3-hop in-closure of M."""
    indeg = np.bincount(dst, minlength=N)
    deg = (1.0 + indeg).astype(np.float32)
    dinv = (1.0 / np.sqrt(deg)).astype(np.float32)
    self_norm = (1.0 / deg).astype(np.float32)

    M = np.unique(np.concatenate([dst[src == cvi], [cvi]]))

    order = np.argsort(dst, kind="stable")
    dst_sorted = dst[order]
    src_sorted = src[order]
    rowptr = np.zeros(N + 1, dtype=np.int64)
    np.cumsum(np.bincount(dst_sorted, minlength=N), out=rowptr[1:])

    def in_edges_of(nodes):
        cs, cd = [], []
        for n in nodes:
            s, e = rowptr[n], rowptr[n + 1]
            cs.append(src_sorted[s:e])
            cd.append(dst_sorted[s:e])
        if cs:
            return np.concatenate(cs), np.concatenate(cd)
        z = np.array([], np.int64)
        return z, z

    L3 = M
    e3s, e3d = in_edges_of(L3)
    L2 = np.unique(np.concatenate([L3, e3s]))
    e2s, e2d = in_edges_of(L2)
    L1 = np.unique(np.concatenate([L2, e2s]))
    e1s, e1d = in_edges_of(L1)
    L0 = np.unique(np.concatenate([L1, e1s]))

    return (M, (L0, L1, L2, L3),
            ((e1s, e1d), (e2s, e2d), (e3s, e3d)), (dinv, self_norm))


def _build_aggT(rows_nodes, cols_nodes, es, ed, dinv, self_norm, nr, ncol):
    """A.T zero-padded to [ncol, nr]: A[r,c] = sum(edge_norm) + self_norm diag."""
    AT = np.zeros((ncol, nr), np.float32)
    r = np.searchsorted(rows_nodes, ed)
    c = np.searchsorted(cols_nodes, es)
    w = dinv[es] * dinv[ed]
    np.add.at(AT, (c, r), w)
    rr = np.arange(len(rows_nodes))
    cc = np.searchsorted(cols_nodes, rows_nodes)
    AT[cc, rr] += self_norm[rows_nodes]
    return AT


def _tile128(a2d, k):
    """[k*128, f] -> [128, k*f] with tile j at columns [j*f, (j+1)*f)."""
    f = a2d.shape[1]
    return np.ascontiguousarray(
        a2d.reshape(k, 128, f).transpose(1, 0, 2).reshape(128, k * f))


def _numpy_fallback(vertices, src, dst, cvi, W1, b1, W2, b2, W3, b3,
                    Wp1, bp1, Wp2, bp2):
    """Identical-math pruned computation in numpy (used only for graphs whose
    closure exceeds the device bucket caps)."""
    N = vertices.shape[0]
    M, levels, edges, (dinv, self_norm) = _prune(N, src, dst, cvi)
    L0, L1, L2, L3 = levels

    def agg(h, rows, cols, es, ed):
        loc_c = np.searchsorted(cols, es)
        loc_r = np.searchsorted(rows, ed)
        out = np.zeros((len(rows), h.shape[1]), np.float32)
        np.add.at(out, loc_r, h[loc_c] * (dinv[es] * dinv[ed])[:, None])
        out += h[np.searchsorted(cols, rows)] * self_norm[rows][:, None]
        return out

    (e1s, e1d), (e2s, e2d), (e3s, e3d) = edges
    t0 = vertices[L0].astype(np.float32) @ W1
    h1 = np.tanh(agg(t0, L1, L0, e1s, e1d) + b1)
    t1 = h1 @ W2
    h2 = np.tanh(agg(t1, L2, L1, e2s, e2d) + b2)
    t2 = h2 @ W3
    h3 = np.tanh(agg(t2, L3, L2, e3s, e3d) + b3)
    p = h3 @ Wp1 + bp1
    p = np.where(p >= 0, p, ALPHA * p)
    s = (p @ Wp2 + bp2)[:, 0]
    s = s - s.max()
    e = np.exp(s)
    out = np.zeros(N, np.float32)
    out[M] = e / e.sum()
    return out


# --------------------------------------------------------------------------
# Entry point
# --------------------------------------------------------------------------

def kernel(**inputs) -> np.ndarray:
    global last_results
    vertices = np.ascontiguousarray(np.asarray(inputs["vertices"], np.float32))
    edge_index = np.asarray(inputs["edge_index"])
    cvi = int(np.asarray(inputs["current_vertex_idx"]))
    W1 = np.asarray(inputs["W1"], np.float32)
    W2 = np.asarray(inputs["W2"], np.float32)
    W3 = np.asarray(inputs["W3"], np.float32)
    Wp1 = np.asarray(inputs["Wp1"], np.float32)
    Wp2 = np.asarray(inputs["Wp2"], np.float32)
    b1 = np.asarray(inputs["b1"], np.float32)
    b2 = np.asarray(inputs["b2"], np.float32)
    b3 = np.asarray(inputs["b3"], np.float32)
    bp1 = np.asarray(inputs["bp1"], np.float32)
    bp2 = np.asarray(inputs["bp2"], np.float32)

    N = vertices.shape[0]
    src = np.asarray(edge_index[0], np.int64)
    dst = np.asarray(edge_index[1], np.int64)

    M, levels, edges, (dinv, self_norm) = _prune(N, src, dst, cvi)
    L0, L1, L2, L3 = levels
    (e1s, e1d), (e2s, e2d), (e3s, e3d) = edges

    n0 = _next_size(len(L0), 128)
    n1 = _next_size(len(L1), 64)
    n2 = _next_size(len(L2), 16)
    n3 = _next_size(len(L3), 8)
    # keep n1/n2 within the lo-half (<=64 partition rows)
    bucket = (n0, n1, n2, n3)
    lenM = len(M)
    if n0 > MAX_N0 or n1 > MAX_N1 or n2 > MAX_N2 or n3 > MAX_N3:
        return _numpy_fallback(vertices, src, dst, cvi, W1, b1, W2, b2,
                               W3, b3, Wp1, bp1, Wp2, bp2)
    k0 = n0 // 128
    zero_bias = bool(not b1.any() and not b2.any() and not b3.any()
                     and not bp1.any())
    key = (bucket, lenM, zero_bias)

    x0 = np.zeros((n0, F_IN), np.float32)
    x0[:len(L0)] = vertices[L0]
    a1t = _build_aggT(L1, L0, e1s, e1d, dinv, self_norm, n1, n0)
    a2t = _build_aggT(L2, L1, e2s, e2d, dinv, self_norm, n2, n1)
    a3t = _build_aggT(L3, L2, e3s, e3d, dinv, self_norm, n3, n2)

    lay = _blob_layout(bucket)
    blob = np.zeros((128, lay["_total"]), np.float32)
    blob[:, lay["x0"]:lay["x0"] + k0 * F_IN] = _tile128(x0, k0)
    blob[:, lay["a1"]:lay["a1"] + k0 * n1] = _tile128(a1t, k0)
    blob[:, lay["b1"]:lay["b1"] + KD] = b1.reshape(KD, 128).T
    blob[:n1, lay["a2"]:lay["a2"] + n2] = a2t
    blob[:, lay["b2"]:lay["b2"] + KD] = b2.reshape(KD, 128).T
    blob[:n2, lay["a3"]:lay["a3"] + n3] = a3t
    blob[:, lay["b3"]:lay["b3"] + KD] = b3.reshape(KD, 128).T
    blob[:, lay["bp1"]:lay["bp1"] + KH] = bp1.reshape(KH, 128).T
    blob[:, lay["wp2"]:lay["wp2"] + KH] = Wp2.reshape(KH, 128).T

    # wp1 packed for DoubleRow: [r, hi*2+p, g, c] = Wp1[(2p+g)*128 + r, hi*128+c]
    wp1r = np.empty((128, KH * 2, 2, 128), np.float32)
    for hi in range(KH):
        for p in range(KD // 2):
            for g in range(2):
                wp1r[:, hi * 2 + p, g, :] = \
                    Wp1[(2 * p + g) * 128:(2 * p + g + 1) * 128,
                        hi * 128:(hi + 1) * 128]

    w23 = np.empty((128, 2, KD, D), np.float32)
    w23[:, 0] = _tile128(W2, KD).reshape(128, KD, D)
    w23[:, 1] = _tile128(W3, KD).reshape(128, KD, D)

    blob16 = blob.astype(bf16)
    in_map = {
        "w1": np.ascontiguousarray(W1.astype(bf16)),
        "Alo": np.ascontiguousarray(blob16[0:64]),
        "Ahi": np.ascontiguousarray(blob16[64:128, :lay["_hi"]]),
        "w23": np.ascontiguousarray(w23.astype(fp8)),
        "wp1": np.ascontiguousarray(wp1r.astype(fp8)),
    }

    import os
    nc = _get_program(key)
    if os.environ.get("BASS_TRACE"):
        # profiling path (test harness): full run_bass_kernel_spmd with NTFF
        from concourse.bass_utils import run_bass_kernel_spmd
        last_results = run_bass_kernel_spmd(
            nc, [in_map] * N_CORES, list(range(N_CORES)))
        probs = np.asarray(last_results.results[0]["out"]).reshape(-1)
    else:
        out_map = _run_fast(key, nc, in_map)
        last_results = ("fast", out_map)
        probs = np.asarray(out_map["out"]).reshape(-1)

    out = np.zeros(N, np.float32)
    out[M] = probs[:lenM]
    return out


# revision 57
# speedup vs baseline: 1.0030x; 1.0030x over previous
"""Trainium2 Bass kernel for a 3-layer GCN + MLP scorer with neighbor-masked softmax.

The reference computes, for a graph with N nodes / E edges:
    h = tanh(GCN(tanh(GCN(tanh(GCN(x)))))); scores = MLP(h)
    out = softmax(scores masked to out-neighbors of current_vertex_idx)

The softmax mask makes the output exactly zero outside M = {out-neighbors of
cvi} | {cvi}.  Only the 3-hop *in*-neighborhood of M (a few hundred nodes of
the 50k) can influence the masked scores, so the kernel prunes the graph to
that closure on the host, builds small dense aggregation matrices (adjacency
with GCN normalization baked in), and runs the entire floating-point
computation on-device as a chain of dense matmuls + activations.  The device
program is SPMD-replicated across the 8 NeuronCores.

Host work is index-only (degree counts, BFS closure, packing the per-call
aggregation matrices); every FLOP of the model runs on the NeuronCores.

Device-side notes (v2):
  - All matmul operands are bf16 (fp32 PSUM accumulate): single-pass matmuls
    (fp32 needs 2 half-speed passes) and half the HBM traffic.  The softmax
    tail stays fp32.
  - Weights stream over both hardware DGE queues (sync + scalar), ordered by
    first use (W2 halves first, Wp1 last); W2/W3 are split into column halves
    so the first half of t = h @ W overlaps the second half's DMA.
  - Layer 1 is reassociated as (A1 @ x0) @ W1 (contract the node dim first at
    F_IN=16 wide); layouts alternate node-major/feature-major so every matmul
    contracts on partitions with no transposes.
  - Softmax is restricted to the first len(M) lanes (padding lanes are never
    read), which removes the -inf mask row; bp2 is dropped (softmax is
    invariant to constant shifts).  Leaky-relu is a single Prelu activation
    with the bias folded in via a K=1 matmul.
"""

import numpy as np
import ml_dtypes

D = 512      # node embedding size
H = 256      # predictor hidden size
F_IN = 16    # raw node feature dim
ALPHA = 0.1  # leaky relu slope
N_CORES = 8
KD = D // 128
KH = H // 128
N_WARMUP = 4  # dummy matmuls that ramp the PE clock during the DMA window

bf16 = ml_dtypes.bfloat16
fp8 = ml_dtypes.float8_e4m3

# Bucket caps: beyond these we fall back to the (identical-math) numpy path.
# n1/n2 <= 64 so the aggregation operands live in partition rows 0-63 (the
# "lo" half of the split input blob).
MAX_N0 = 4096
MAX_N1 = 64
MAX_N2 = 64
MAX_N3 = 64

_prog_cache: dict[tuple, object] = {}
last_results = None  # BassKernelResults of the most recent device run


def _blob_layout(bucket):
    """Column layout of the input blob A (bf16).  Columns [0, _hi) are
    needed on all 128 partition rows and are DMA'd as two row-halves (the
    DGE is descriptor-rate-bound, one descriptor per row, so row-splitting
    across both queues halves the wall time).  Columns [_hi, _total) only
    ever feed partition rows < 64 (w1 uses 16 rows, a2/a3 use n1/n2 <= 64)
    and ride only in the lo-half parameter."""
    n0, n1, n2, n3 = bucket
    k0 = n0 // 128
    off = 0
    lay = {}
    lay["x0"] = off; off += k0 * F_IN
    lay["a1"] = off; off += k0 * n1
    lay["b1"] = off; off += KD
    lay["b2"] = off; off += KD
    lay["b3"] = off; off += KD
    lay["bp1"] = off; off += KH     # bp1 feature-major columns
    lay["wp2"] = off; off += KH     # feature-major columns
    lay["_hi"] = off                # hi-half parameter covers [0, _hi)
    lay["w1f"] = off; off += D      # W1 [16, 512] in partition rows 0-15
    lay["a2"] = off; off += n2      # rows 0..n1
    lay["a3"] = off; off += n3      # rows 0..n2
    lay["_total"] = off
    return lay


# --------------------------------------------------------------------------
# Device program
# --------------------------------------------------------------------------

def _build_program(key):
    import concourse.bass as bass
    import concourse.bacc as bacc
    import concourse.mybir as mybir
    import concourse.tile as tile

    bucket, lenM, zero_bias = key
    n0, n1, n2, n3 = bucket
    f32 = mybir.dt.float32
    b16 = mybir.dt.bfloat16
    f8 = mybir.dt.float8e4
    k0 = n0 // 128
    Tanh = mybir.ActivationFunctionType.Tanh
    Prelu = mybir.ActivationFunctionType.Prelu
    Exp = mybir.ActivationFunctionType.Exp
    Copy = mybir.ActivationFunctionType.Copy
    DR = mybir.MatmulPerfMode.DoubleRow
    lay = _blob_layout(bucket)
    CA = lay["_total"]
    CH = lay["_hi"]
    DH = D // 2   # 256: t1/t2 PSUM->SBUF cast half width

    nc = bacc.Bacc("TRN2", target_bir_lowering=False, debug=False)
    P_Alo = nc.declare_dram_parameter("Alo", [64, CA], b16, isOutput=False)
    P_Ahi = nc.declare_dram_parameter("Ahi", [64, CH], b16, isOutput=False)
    P_w23 = nc.declare_dram_parameter("w23", [128, 2, KD, D], f8,
                                      isOutput=False)
    P_wp1 = nc.declare_dram_parameter("wp1", [128, KH * 2, 2, 128], f8,
                                      isOutput=False)
    P_out = nc.declare_dram_parameter("out", [1, lenM], f32, isOutput=True)

    with tile.TileContext(nc) as tc:
        with (
            tc.tile_pool(name="sb", bufs=1) as sb,
            tc.tile_pool(name="ps", bufs=4, space="PSUM") as ps,
            tc.tile_pool(name="pss", bufs=2, space="PSUM") as pss,
            tc.tile_pool(name="wu", bufs=1, space="PSUM") as wu,
        ):
            # ---- input DMAs: the DGE is descriptor-rate-bound (one
            # descriptor per partition row), so the blob A (whose delivery
            # gates the whole H1 chain) is row-split across both queues;
            # W2|W3 (4KB descriptors) fill the sync queue's second slot and
            # wp1 the scalar queue's.
            A = sb.tile([128, CA], b16, tag="A")
            nc.sync.dma_start(A[0:64, :], P_Alo[:])
            nc.scalar.dma_start(A[64:128, 0:CH], P_Ahi[:])
            w23 = sb.tile([128, 2, KD, D], f8, tag="w23")
            nc.sync.dma_start(w23[:], P_w23[:])
            wp1 = sb.tile([128, KH * 2, 2, 128], f8, tag="wp1")
            nc.scalar.dma_start(wp1[:], P_wp1[:])

            # ---- PE warm-up: the tensor engine's clock ramps with use (low ->
            # mid p-state); a fat dummy absorbs the ramp and short keep-alives
            # hold the clock until the first real matmul.
            wu_sb = sb.tile([128, 512], b16, tag="wu_sb")
            nc.vector.memset(wu_sb[:], 0)
            wu_ps = wu.tile([128, 512], f32, tag="wu_ps")
            for i in range(N_WARMUP):
                w_cols = 512 if i < 1 else 64
                nc.tensor.matmul(wu_ps[:, 0:w_cols], wu_sb[:, 0:128],
                                 wu_sb[:, 0:w_cols], start=True, stop=True)
            wu_out = sb.tile([1, 1], f32, tag="wu_out")
            nc.vector.tensor_copy(wu_out[:], wu_ps[0:1, 0:1])

            def acol(name, i=0, w=1, p0=0, p1=128):
                return A[p0:p1, lay[name] + i:lay[name] + i + w]

            def act_bias(out_ap, acc_ap, func, bname, di, **kw):
                if zero_bias:
                    nc.scalar.activation(out_ap, acc_ap, func, **kw)
                else:
                    nc.scalar.activation(out_ap, acc_ap, func,
                                         bias=acol(bname, di), **kw)

            # ---- BT = (A1 @ x0).T : [F_IN, n1]  (contract n0 nodes)
            acc = pss.tile([F_IN, n1], f32, tag="pss")
            for j in range(k0):
                nc.tensor.matmul(acc[:], acol("x0", j * F_IN, F_IN),
                                 acol("a1", j * n1, n1),
                                 start=(j == 0), stop=(j == k0 - 1))
            bt = sb.tile([F_IN, n1], b16, tag="bt")
            nc.vector.tensor_copy(bt[:], acc[:])

            # ---- H1[:, di] = tanh(W1[:, dslice].T @ BT + b1) : feature-major
            # fp8 so the dense layers can run DoubleRow (2 K-rows/cycle).
            # Tanhs run per d-tile PAIR (separate PSUM tiles per pair: a
            # group's start=True zeroes bank-granular regions, so mixing
            # groups and readers in one bank serializes them).
            H1 = sb.tile([128, KD, n1], f8, tag="H1")
            for pr in range(KD // 2):
                accp2 = ps.tile([128, 2, n1], f32, tag="ps")
                for g in range(2):
                    di = 2 * pr + g
                    nc.tensor.matmul(accp2[:, g, :],
                                     acol("w1f", di * 128, 128, 0, F_IN),
                                     bt[:], start=True, stop=True)
                    if not zero_bias:
                        act_bias(H1[:, di, :], accp2[:, g, :], Tanh, "b1", di)
                if zero_bias:
                    nc.scalar.activation(H1[:, 2 * pr:2 * pr + 2, :],
                                         accp2[:], Tanh)

            def dense_layer(Hf, n_rows, wi, lname):
                # t = h @ W : node-major [n_rows, D] via fp8 DoubleRow pairs;
                # PSUM->SBUF bf16 cast halves on vector+scalar engines.
                t_sb = sb.tile([n_rows, D], b16, tag=lname)
                acc = ps.tile([n_rows, D], f32, tag="ps")
                for p in range(KD // 2):
                    nc.tensor.matmul(acc[:], Hf[:, 2 * p:2 * p + 2, :],
                                     w23[:, wi, 2 * p:2 * p + 2, :],
                                     start=(p == 0), stop=(p == KD // 2 - 1),
                                     perf_mode=DR)
                for q in range(4):
                    qs = slice(q * 128, (q + 1) * 128)
                    nc.vector.tensor_copy(t_sb[:, qs], acc[:, qs])
                return t_sb

            def agg_layer(t_in, n_in, aname, n_out, bname, lname):
                # Hf[:, di] = tanh((A @ t).T + b) : feature-major, fp8,
                # per-pair PSUM tiles + tanh so t(next) pair 0 starts early
                Hf = sb.tile([128, KD, n_out], f8, tag=lname)
                for pr in range(KD // 2):
                    accp2 = ps.tile([128, 2, n_out], f32, tag="ps")
                    for g in range(2):
                        di = 2 * pr + g
                        nc.tensor.matmul(accp2[:, g, :],
                                         t_in[:, di * 128:(di + 1) * 128],
                                         acol(aname, 0, n_out, 0, n_in),
                                         start=True, stop=True)
                        if not zero_bias:
                            act_bias(Hf[:, di, :], accp2[:, g, :], Tanh,
                                     bname, di)
                    if zero_bias:
                        nc.scalar.activation(Hf[:, 2 * pr:2 * pr + 2, :],
                                             accp2[:], Tanh)
                return Hf

            t1 = dense_layer(H1, n1, 0, "t1")
            H2 = agg_layer(t1, n1, "a2", n2, "b2", "H2")
            t2 = dense_layer(H2, n2, 1, "t2")
            H3 = agg_layer(t2, n2, "a3", n3, "b3", "H3")

            # ---- predictor hidden: P[:, hi] = prelu(Wp1.T @ h3.T + bp1)
            # (fp8 DoubleRow over d-tile pairs; bp1 rides the activation bias)
            Pf = sb.tile([128, KH * n3], b16, tag="Pf")
            for hi in range(KH):
                accp = ps.tile([128, n3], f32, tag="ps")
                for p in range(KD // 2):
                    nc.tensor.matmul(
                        accp[:], wp1[:, hi * 2 + p, :, :],
                        H3[:, 2 * p:2 * p + 2, :], start=(p == 0),
                        stop=(p == KD // 2 - 1), perf_mode=DR)
                act_bias(Pf[:, hi * n3:(hi + 1) * n3], accp[:],
                         Prelu, "bp1", hi, alpha=ALPHA)

            # ---- scores (first lenM lanes only; softmax is shift-invariant
            # so bp2 is dropped, and padding lanes are never read)
            acc = pss.tile([1, n3], f32, tag="pss")
            for hi in range(KH):
                nc.tensor.matmul(acc[:], acol("wp2", hi),
                                 Pf[:, hi * n3:(hi + 1) * n3],
                                 start=(hi == 0), stop=(hi == KH - 1))
            e = sb.tile([1, lenM], f32, tag="e")
            nc.scalar.activation(e[:], acc[0:1, 0:lenM], Exp)
            ssum = sb.tile([1, 1], f32, tag="ssum")
            nc.vector.tensor_reduce(ssum[:], e[:], mybir.AxisListType.X,
                                    mybir.AluOpType.add)
            rs = sb.tile([1, 1], f32, tag="rs")
            nc.vector.reciprocal(rs[:], ssum[:])
            o = sb.tile([1, lenM], f32, tag="o")
            nc.vector.tensor_scalar_mul(o[:], e[:], rs[:])
            nc.sync.dma_start(P_out[:], o[:])

    nc.compile()
    return nc


def _get_program(key):
    prog = _prog_cache.get(key)
    if prog is None:
        prog = _build_program(key)
        _prog_cache[key] = prog
    return prog


_runner_cache: dict[tuple, dict] = {}
_dev_weights: dict[str, tuple] = {}
# inputs that rarely change between calls: keep them resident on-device
_WEIGHT_PARAMS = ("w23", "wp1")


def _get_runner(key, nc):
    """Compile-once executor for the SPMD program (the per-call jit rebuild in
    run_bass_kernel_spmd re-traces and re-compiles; this caches the compiled
    shard_map callable per bucket)."""
    r = _runner_cache.get(key)
    if r is not None:
        return r
    import jax
    import numpy as np
    from jax.sharding import Mesh, PartitionSpec
    from jax.experimental.shard_map import shard_map
    from concourse import bass2jax
    import concourse.mybir as mybir

    bass2jax.install_neuronx_cc_hook()
    partition_name = (nc.partition_id_tensor.name
                      if nc.partition_id_tensor else None)
    in_names, out_names, out_avals = [], [], []
    for alloc in nc.m.functions[0].allocations:
        if not isinstance(alloc, mybir.MemoryLocationSet):
            continue
        name = alloc.memorylocations[0].name
        if alloc.kind == "ExternalInput":
            if name != partition_name:
                in_names.append(name)
        elif alloc.kind == "ExternalOutput":
            out_names.append(name)
            out_avals.append(jax.core.ShapedArray(
                tuple(alloc.tensor_shape), mybir.dt.np(alloc.dtype)))
    n_params = len(in_names)
    all_names = in_names + out_names
    if partition_name is not None:
        all_names = all_names + [partition_name]
    all_names = tuple(all_names)

    def _body(*args):
        operands = list(args)
        if partition_name is not None:
            operands.append(bass2jax.partition_id_tensor())
        outs = bass2jax._bass_exec_p.bind(
            *operands, out_avals=tuple(out_avals), in_names=all_names,
            out_names=tuple(out_names), lowering_input_output_aliases=(),
            sim_require_finite=True, sim_require_nnan=True, nc=nc)
        return tuple(outs)

    devices = jax.devices()[:N_CORES]
    mesh = Mesh(np.asarray(devices), ("core",))
    in_specs = (PartitionSpec("core"),) * (n_params + len(out_names))
    out_specs = (PartitionSpec("core"),) * len(out_names)
    donate = tuple(range(n_params, n_params + len(out_names)))
    fn = jax.jit(
        shard_map(_body, mesh=mesh, in_specs=in_specs, out_specs=out_specs,
                  check_rep=False),
        donate_argnums=donate, keep_unused=True)
    r = dict(fn=fn, in_names=in_names, out_names=out_names,
             out_avals=out_avals, mesh=mesh)
    _runner_cache[key] = r
    return r


def _run_fast(key, nc, in_map):
    """Execute via the cached runner; returns core-0 output dict."""
    import jax
    from jax.sharding import NamedSharding, PartitionSpec

    r = _get_runner(key, nc)
    sharding = NamedSharding(r["mesh"], PartitionSpec("core"))
    args = []
    for name in r["in_names"]:
        arr = np.ascontiguousarray(in_map[name])
        if name in _WEIGHT_PARAMS:
            cached = _dev_weights.get(name)
            if cached is not None and cached[0].shape == arr.shape and \
                    np.array_equal(cached[0], arr):
                args.append(cached[1])
                continue
            dev = jax.device_put(
                np.concatenate([arr] * N_CORES, axis=0), sharding)
            _dev_weights[name] = (arr.copy(), dev)
            args.append(dev)
        else:
            args.append(np.concatenate([arr] * N_CORES, axis=0))
    zeros = [np.zeros((N_CORES * a.shape[0], *a.shape[1:]), a.dtype)
             for a in r["out_avals"]]
    outs = r["fn"](*args, *zeros)
    return {
        name: np.asarray(outs[i]).reshape(N_CORES, *r["out_avals"][i].shape)[0]
        for i, name in enumerate(r["out_names"])
    }


# --------------------------------------------------------------------------
# Host-side graph pruning / packing
# --------------------------------------------------------------------------

def _next_size(n, minimum):
    r = minimum
    while r < n:
        r *= 2
    return r


def _prune(N, src, dst, cvi):
    """Return (M, levels, edges, norms) for the 3-hop in-closure of M."""
    indeg = np.bincount(dst, minlength=N)
    deg = (1.0 + indeg).astype(np.float32)
    dinv = (1.0 / np.sqrt(deg)).astype(np.float32)
    self_norm = (1.0 / deg).astype(np.float32)

    M = np.unique(np.concatenate([dst[src == cvi], [cvi]]))

    order = np.argsort(dst, kind="stable")
    dst_sorted = dst[order]
    src_sorted = src[order]
    rowptr = np.zeros(N + 1, dtype=np.int64)
    np.cumsum(np.bincount(dst_sorted, minlength=N), out=rowptr[1:])

    def in_edges_of(nodes):
        cs, cd = [], []
        for n in nodes:
            s, e = rowptr[n], rowptr[n + 1]
            cs.append(src_sorted[s:e])
            cd.append(dst_sorted[s:e])
        if cs:
            return np.concatenate(cs), np.concatenate(cd)
        z = np.array([], np.int64)
        return z, z

    L3 = M
    e3s, e3d = in_edges_of(L3)
    L2 = np.unique(np.concatenate([L3, e3s]))
    e2s, e2d = in_edges_of(L2)
    L1 = np.unique(np.concatenate([L2, e2s]))
    e1s, e1d = in_edges_of(L1)
    L0 = np.unique(np.concatenate([L1, e1s]))

    return (M, (L0, L1, L2, L3),
            ((e1s, e1d), (e2s, e2d), (e3s, e3d)), (dinv, self_norm))


def _build_aggT(rows_nodes, cols_nodes, es, ed, dinv, self_norm, nr, ncol):
    """A.T zero-padded to [ncol, nr]: A[r,c] = sum(edge_norm) + self_norm diag."""
    AT = np.zeros((ncol, nr), np.float32)
    r = np.searchsorted(rows_nodes, ed)
    c = np.searchsorted(cols_nodes, es)
    w = dinv[es] * dinv[ed]
    np.add.at(AT, (c, r), w)
    rr = np.arange(len(rows_nodes))
    cc = np.searchsorted(cols_nodes, rows_nodes)
    AT[cc, rr] += self_norm[rows_nodes]
    return AT


def _tile128(a2d, k):
    """[k*128, f] -> [128, k*f] with tile j at columns [j*f, (j+1)*f)."""
    f = a2d.shape[1]
    return np.ascontiguousarray(
        a2d.reshape(k, 128, f).transpose(1, 0, 2).reshape(128, k * f))


def _numpy_fallback(vertices, src, dst, cvi, W1, b1, W2, b2, W3, b3,
                    Wp1, bp1, Wp2, bp2):
    """Identical-math pruned computation in numpy (used only for graphs whose
    closure exceeds the device bucket caps)."""
    N = vertices.shape[0]
    M, levels, edges, (dinv, self_norm) = _prune(N, src, dst, cvi)
    L0, L1, L2, L3 = levels

    def agg(h, rows, cols, es, ed):
        loc_c = np.searchsorted(cols, es)
        loc_r = np.searchsorted(rows, ed)
        out = np.zeros((len(rows), h.shape[1]), np.float32)
        np.add.at(out, loc_r, h[loc_c] * (dinv[es] * dinv[ed])[:, None])
        out += h[np.searchsorted(cols, rows)] * self_norm[rows][:, None]
        return out

    (e1s, e1d), (e2s, e2d), (e3s, e3d) = edges
    t0 = vertices[L0].astype(np.float32) @ W1
    h1 = np.tanh(agg(t0, L1, L0, e1s, e1d) + b1)
    t1 = h1 @ W2
    h2 = np.tanh(agg(t1, L2, L1, e2s, e2d) + b2)
    t2 = h2 @ W3
    h3 = np.tanh(agg(t2, L3, L2, e3s, e3d) + b3)
    p = h3 @ Wp1 + bp1
    p = np.where(p >= 0, p, ALPHA * p)
    s = (p @ Wp2 + bp2)[:, 0]
    s = s - s.max()
    e = np.exp(s)
    out = np.zeros(N, np.float32)
    out[M] = e / e.sum()
    return out


# --------------------------------------------------------------------------
# Entry point
# --------------------------------------------------------------------------

def kernel(**inputs) -> np.ndarray:
    global last_results
    vertices = np.ascontiguousarray(np.asarray(inputs["vertices"], np.float32))
    edge_index = np.asarray(inputs["edge_index"])
    cvi = int(np.asarray(inputs["current_vertex_idx"]))
    W1 = np.asarray(inputs["W1"], np.float32)
    W2 = np.asarray(inputs["W2"], np.float32)
    W3 = np.asarray(inputs["W3"], np.float32)
    Wp1 = np.asarray(inputs["Wp1"], np.float32)
    Wp2 = np.asarray(inputs["Wp2"], np.float32)
    b1 = np.asarray(inputs["b1"], np.float32)
    b2 = np.asarray(inputs["b2"], np.float32)
    b3 = np.asarray(inputs["b3"], np.float32)
    bp1 = np.asarray(inputs["bp1"], np.float32)
    bp2 = np.asarray(inputs["bp2"], np.float32)

    N = vertices.shape[0]
    src = np.asarray(edge_index[0], np.int64)
    dst = np.asarray(edge_index[1], np.int64)

    M, levels, edges, (dinv, self_norm) = _prune(N, src, dst, cvi)
    L0, L1, L2, L3 = levels
    (e1s, e1d), (e2s, e2d), (e3s, e3d) = edges

    n0 = _next_size(len(L0), 128)
    n1 = _next_size(len(L1), 64)
    n2 = _next_size(len(L2), 16)
    n3 = _next_size(len(L3), 8)
    # keep n1/n2 within the lo-half (<=64 partition rows)
    bucket = (n0, n1, n2, n3)
    lenM = len(M)
    if n0 > MAX_N0 or n1 > MAX_N1 or n2 > MAX_N2 or n3 > MAX_N3:
        return _numpy_fallback(vertices, src, dst, cvi, W1, b1, W2, b2,
                               W3, b3, Wp1, bp1, Wp2, bp2)
    k0 = n0 // 128
    zero_bias = bool(not b1.any() and not b2.any() and not b3.any()
                     and not bp1.any())
    key = (bucket, lenM, zero_bias)

    x0 = np.zeros((n0, F_IN), np.float32)
    x0[:len(L0)] = vertices[L0]
    a1t = _build_aggT(L1, L0, e1s, e1d, dinv, self_norm, n1, n0)
    a2t = _build_aggT(L2, L1, e2s, e2d, dinv, self_norm, n2, n1)
    a3t = _build_aggT(L3, L2, e3s, e3d, dinv, self_norm, n3, n2)

    lay = _blob_layout(bucket)
    blob = np.zeros((128, lay["_total"]), np.float32)
    blob[:, lay["x0"]:lay["x0"] + k0 * F_IN] = _tile128(x0, k0)
    blob[:, lay["a1"]:lay["a1"] + k0 * n1] = _tile128(a1t, k0)
    blob[:, lay["b1"]:lay["b1"] + KD] = b1.reshape(KD, 128).T
    blob[:n1, lay["a2"]:lay["a2"] + n2] = a2t
    blob[:, lay["b2"]:lay["b2"] + KD] = b2.reshape(KD, 128).T
    blob[:n2, lay["a3"]:lay["a3"] + n3] = a3t
    blob[:, lay["b3"]:lay["b3"] + KD] = b3.reshape(KD, 128).T
    blob[:, lay["bp1"]:lay["bp1"] + KH] = bp1.reshape(KH, 128).T
    blob[:, lay["wp2"]:lay["wp2"] + KH] = Wp2.reshape(KH, 128).T
    blob[:F_IN, lay["w1f"]:lay["w1f"] + D] = W1

    # wp1 packed for DoubleRow: [r, hi*2+p, g, c] = Wp1[(2p+g)*128 + r, hi*128+c]
    wp1r = np.empty((128, KH * 2, 2, 128), np.float32)
    for hi in range(KH):
        for p in range(KD // 2):
            for g in range(2):
                wp1r[:, hi * 2 + p, g, :] = \
                    Wp1[(2 * p + g) * 128:(2 * p + g + 1) * 128,
                        hi * 128:(hi + 1) * 128]

    w23 = np.empty((128, 2, KD, D), np.float32)
    w23[:, 0] = _tile128(W2, KD).reshape(128, KD, D)
    w23[:, 1] = _tile128(W3, KD).reshape(128, KD, D)

    blob16 = blob.astype(bf16)
    in_map = {
        "Alo": np.ascontiguousarray(blob16[0:64]),
        "Ahi": np.ascontiguousarray(blob16[64:128, :lay["_hi"]]),
        "w23": np.ascontiguousarray(w23.astype(fp8)),
        "wp1": np.ascontiguousarray(wp1r.astype(fp8)),
    }

    import os
    nc = _get_program(key)
    if os.environ.get("BASS_TRACE"):
        # profiling path (test harness): full run_bass_kernel_spmd with NTFF
        from concourse.bass_utils import run_bass_kernel_spmd
        last_results = run_bass_kernel_spmd(
            nc, [in_map] * N_CORES, list(range(N_CORES)))
        probs = np.asarray(last_results.results[0]["out"]).reshape(-1)
    else:
        out_map = _run_fast(key, nc, in_map)
        last_results = ("fast", out_map)
        probs = np.asarray(out_map["out"]).reshape(-1)

    out = np.zeros(N, np.float32)
    out[M] = probs[:lenM]
    return out


# revision 58
# speedup vs baseline: 1.0255x; 1.0225x over previous
"""Trainium2 Bass kernel for a 3-layer GCN + MLP scorer with neighbor-masked softmax.

The reference computes, for a graph with N nodes / E edges:
    h = tanh(GCN(tanh(GCN(tanh(GCN(x)))))); scores = MLP(h)
    out = softmax(scores masked to out-neighbors of current_vertex_idx)

The softmax mask makes the output exactly zero outside M = {out-neighbors of
cvi} | {cvi}.  Only the 3-hop *in*-neighborhood of M (a few hundred nodes of
the 50k) can influence the masked scores, so the kernel prunes the graph to
that closure on the host, builds small dense aggregation matrices (adjacency
with GCN normalization baked in), and runs the entire floating-point
computation on-device as a chain of dense matmuls + activations.  The device
program is SPMD-replicated across the 8 NeuronCores.

Host work is index-only (degree counts, BFS closure, packing the per-call
aggregation matrices); every FLOP of the model runs on the NeuronCores.

Device-side notes (v2):
  - All matmul operands are bf16 (fp32 PSUM accumulate): single-pass matmuls
    (fp32 needs 2 half-speed passes) and half the HBM traffic.  The softmax
    tail stays fp32.
  - Weights stream over both hardware DGE queues (sync + scalar), ordered by
    first use (W2 halves first, Wp1 last); W2/W3 are split into column halves
    so the first half of t = h @ W overlaps the second half's DMA.
  - Layer 1 is reassociated as (A1 @ x0) @ W1 (contract the node dim first at
    F_IN=16 wide); layouts alternate node-major/feature-major so every matmul
    contracts on partitions with no transposes.
  - Softmax is restricted to the first len(M) lanes (padding lanes are never
    read), which removes the -inf mask row; bp2 is dropped (softmax is
    invariant to constant shifts).  Leaky-relu is a single Prelu activation
    with the bias folded in via a K=1 matmul.
"""

import numpy as np
import ml_dtypes

D = 512      # node embedding size
H = 256      # predictor hidden size
F_IN = 16    # raw node feature dim
ALPHA = 0.1  # leaky relu slope
N_CORES = 8
KD = D // 128
KH = H // 128
N_WARMUP = 10  # dummy matmuls that ramp the PE clock during the DMA window

bf16 = ml_dtypes.bfloat16
fp8 = ml_dtypes.float8_e4m3

# Bucket caps: beyond these we fall back to the (identical-math) numpy path.
# n1/n2 <= 64 so the aggregation operands live in partition rows 0-63 (the
# "lo" half of the split input blob).
MAX_N0 = 4096
MAX_N1 = 64
MAX_N2 = 64
MAX_N3 = 64

_prog_cache: dict[tuple, object] = {}
last_results = None  # BassKernelResults of the most recent device run


def _blob_layout(bucket):
    """Column layout of the input blob A (bf16).  Columns [0, _hi) are
    needed on all 128 partition rows and are DMA'd as two row-halves (the
    DGE is descriptor-rate-bound, one descriptor per row, so row-splitting
    across both queues halves the wall time).  Columns [_hi, _total) only
    ever feed partition rows < 64 (w1 uses 16 rows, a2/a3 use n1/n2 <= 64)
    and ride only in the lo-half parameter."""
    n0, n1, n2, n3 = bucket
    k0 = n0 // 128
    off = 0
    lay = {}
    lay["x0"] = off; off += k0 * F_IN
    lay["a1"] = off; off += k0 * n1
    lay["b1"] = off; off += KD
    lay["b2"] = off; off += KD
    lay["b3"] = off; off += KD
    lay["bp1"] = off; off += KH     # bp1 feature-major columns
    lay["wp2"] = off; off += KH     # feature-major columns
    lay["_hi"] = off                # hi-half parameter covers [0, _hi)
    lay["w1f"] = off; off += D      # W1 [16, 512] in partition rows 0-15
    lay["a2"] = off; off += n2      # rows 0..n1
    lay["a3"] = off; off += n3      # rows 0..n2
    lay["_total"] = off
    return lay


# --------------------------------------------------------------------------
# Device program
# --------------------------------------------------------------------------

def _build_program(key):
    import concourse.bass as bass
    import concourse.bacc as bacc
    import concourse.mybir as mybir
    import concourse.tile as tile

    bucket, lenM, zero_bias = key
    n0, n1, n2, n3 = bucket
    f32 = mybir.dt.float32
    b16 = mybir.dt.bfloat16
    f8 = mybir.dt.float8e4
    k0 = n0 // 128
    Tanh = mybir.ActivationFunctionType.Tanh
    Prelu = mybir.ActivationFunctionType.Prelu
    Exp = mybir.ActivationFunctionType.Exp
    Copy = mybir.ActivationFunctionType.Copy
    DR = mybir.MatmulPerfMode.DoubleRow
    lay = _blob_layout(bucket)
    CA = lay["_total"]
    CH = lay["_hi"]
    DH = D // 2   # 256: t1/t2 PSUM->SBUF cast half width

    nc = bacc.Bacc("TRN2", target_bir_lowering=False, debug=False)
    P_Alo = nc.declare_dram_parameter("Alo", [64, CA], b16, isOutput=False)
    P_Ahi = nc.declare_dram_parameter("Ahi", [64, CH], b16, isOutput=False)
    P_w23 = nc.declare_dram_parameter("w23", [128, 2, KD, D], f8,
                                      isOutput=False)
    P_wp1 = nc.declare_dram_parameter("wp1", [128, KH * 2, 2, 128], f8,
                                      isOutput=False)
    P_out = nc.declare_dram_parameter("out", [1, lenM], f32, isOutput=True)

    with tile.TileContext(nc) as tc:
        with (
            tc.tile_pool(name="sb", bufs=1) as sb,
            tc.tile_pool(name="ps", bufs=4, space="PSUM") as ps,
            tc.tile_pool(name="pss", bufs=2, space="PSUM") as pss,
            tc.tile_pool(name="wu", bufs=1, space="PSUM") as wu,
        ):
            # ---- input DMAs: the DGE is descriptor-rate-bound (one
            # descriptor per partition row), so the blob A (whose delivery
            # gates the whole H1 chain) is row-split across both queues;
            # W2|W3 (4KB descriptors) fill the sync queue's second slot and
            # wp1 the scalar queue's.
            A = sb.tile([128, CA], b16, tag="A")
            nc.sync.dma_start(A[0:64, :], P_Alo[:])
            nc.scalar.dma_start(A[64:128, 0:CH], P_Ahi[:])
            w23 = sb.tile([128, 2, KD, D], f8, tag="w23")
            nc.sync.dma_start(w23[:], P_w23[:])
            wp1 = sb.tile([128, KH * 2, 2, 128], f8, tag="wp1")
            nc.scalar.dma_start(wp1[:], P_wp1[:])

            # ---- PE warm-up: the tensor engine's clock ramps with use (low ->
            # mid p-state); a fat dummy absorbs the ramp and short keep-alives
            # hold the clock until the first real matmul.
            wu_sb = sb.tile([128, 512], b16, tag="wu_sb")
            nc.vector.memset(wu_sb[:], 0)
            wu_ps = wu.tile([128, 512], f32, tag="wu_ps")
            for i in range(N_WARMUP):
                w_cols = 512 if i < 1 else 64
                nc.tensor.matmul(wu_ps[:, 0:w_cols], wu_sb[:, 0:128],
                                 wu_sb[:, 0:w_cols], start=True, stop=True)
            wu_out = sb.tile([1, 1], f32, tag="wu_out")
            nc.vector.tensor_copy(wu_out[:], wu_ps[0:1, 0:1])

            def acol(name, i=0, w=1, p0=0, p1=128):
                return A[p0:p1, lay[name] + i:lay[name] + i + w]

            def act_bias(out_ap, acc_ap, func, bname, di, **kw):
                if zero_bias:
                    nc.scalar.activation(out_ap, acc_ap, func, **kw)
                else:
                    nc.scalar.activation(out_ap, acc_ap, func,
                                         bias=acol(bname, di), **kw)

            # ---- BT = (A1 @ x0).T : [F_IN, n1]  (contract n0 nodes)
            acc = pss.tile([F_IN, n1], f32, tag="pss")
            for j in range(k0):
                nc.tensor.matmul(acc[:], acol("x0", j * F_IN, F_IN),
                                 acol("a1", j * n1, n1),
                                 start=(j == 0), stop=(j == k0 - 1))
            bt = sb.tile([F_IN, n1], b16, tag="bt")
            nc.vector.tensor_copy(bt[:], acc[:])

            # ---- H1[:, di] = tanh(W1[:, dslice].T @ BT + b1) : feature-major
            # fp8 so the dense layers can run DoubleRow (2 K-rows/cycle).
            # Tanhs run per d-tile PAIR (separate PSUM tiles per pair: a
            # group's start=True zeroes bank-granular regions, so mixing
            # groups and readers in one bank serializes them).
            H1 = sb.tile([128, KD, n1], f8, tag="H1")
            for pr in range(KD // 2):
                accp2 = ps.tile([128, 2, n1], f32, tag="ps")
                for g in range(2):
                    di = 2 * pr + g
                    nc.tensor.matmul(accp2[:, g, :],
                                     acol("w1f", di * 128, 128, 0, F_IN),
                                     bt[:], start=True, stop=True)
                    if not zero_bias:
                        act_bias(H1[:, di, :], accp2[:, g, :], Tanh, "b1", di)
                if zero_bias:
                    nc.scalar.activation(H1[:, 2 * pr:2 * pr + 2, :],
                                         accp2[:], Tanh)

            def dense_layer(Hf, n_rows, wi, lname):
                # t = h @ W : node-major [n_rows, D] via fp8 DoubleRow pairs;
                # PSUM->SBUF bf16 cast halves on vector+scalar engines.
                t_sb = sb.tile([n_rows, D], b16, tag=lname)
                acc = ps.tile([n_rows, D], f32, tag="ps")
                for p in range(KD // 2):
                    nc.tensor.matmul(acc[:], Hf[:, 2 * p:2 * p + 2, :],
                                     w23[:, wi, 2 * p:2 * p + 2, :],
                                     start=(p == 0), stop=(p == KD // 2 - 1),
                                     perf_mode=DR)
                for q in range(4):
                    qs = slice(q * 128, (q + 1) * 128)
                    nc.vector.tensor_copy(t_sb[:, qs], acc[:, qs])
                return t_sb

            def agg_layer(t_in, n_in, aname, n_out, bname, lname):
                # Hf[:, di] = tanh((A @ t).T + b) : feature-major, fp8,
                # per-pair PSUM tiles + tanh so t(next) pair 0 starts early
                Hf = sb.tile([128, KD, n_out], f8, tag=lname)
                for pr in range(KD // 2):
                    accp2 = ps.tile([128, 2, n_out], f32, tag="ps")
                    for g in range(2):
                        di = 2 * pr + g
                        nc.tensor.matmul(accp2[:, g, :],
                                         t_in[:, di * 128:(di + 1) * 128],
                                         acol(aname, 0, n_out, 0, n_in),
                                         start=True, stop=True)
                        if not zero_bias:
                            act_bias(Hf[:, di, :], accp2[:, g, :], Tanh,
                                     bname, di)
                    if zero_bias:
                        nc.scalar.activation(Hf[:, 2 * pr:2 * pr + 2, :],
                                             accp2[:], Tanh)
                return Hf

            t1 = dense_layer(H1, n1, 0, "t1")
            H2 = agg_layer(t1, n1, "a2", n2, "b2", "H2")
            t2 = dense_layer(H2, n2, 1, "t2")
            H3 = agg_layer(t2, n2, "a3", n3, "b3", "H3")

            # ---- predictor hidden: P[:, hi] = prelu(Wp1.T @ h3.T + bp1)
            # (fp8 DoubleRow over d-tile pairs; bp1 rides the activation bias)
            Pf = sb.tile([128, KH * n3], b16, tag="Pf")
            for hi in range(KH):
                accp = ps.tile([128, n3], f32, tag="ps")
                for p in range(KD // 2):
                    nc.tensor.matmul(
                        accp[:], wp1[:, hi * 2 + p, :, :],
                        H3[:, 2 * p:2 * p + 2, :], start=(p == 0),
                        stop=(p == KD // 2 - 1), perf_mode=DR)
                act_bias(Pf[:, hi * n3:(hi + 1) * n3], accp[:],
                         Prelu, "bp1", hi, alpha=ALPHA)

            # ---- scores (first lenM lanes only; softmax is shift-invariant
            # so bp2 is dropped, and padding lanes are never read)
            acc = pss.tile([1, n3], f32, tag="pss")
            for hi in range(KH):
                nc.tensor.matmul(acc[:], acol("wp2", hi),
                                 Pf[:, hi * n3:(hi + 1) * n3],
                                 start=(hi == 0), stop=(hi == KH - 1))
            e = sb.tile([1, lenM], f32, tag="e")
            nc.scalar.activation(e[:], acc[0:1, 0:lenM], Exp)
            ssum = sb.tile([1, 1], f32, tag="ssum")
            nc.vector.tensor_reduce(ssum[:], e[:], mybir.AxisListType.X,
                                    mybir.AluOpType.add)
            rs = sb.tile([1, 1], f32, tag="rs")
            nc.vector.reciprocal(rs[:], ssum[:])
            o = sb.tile([1, lenM], f32, tag="o")
            nc.vector.tensor_scalar_mul(o[:], e[:], rs[:])
            nc.sync.dma_start(P_out[:], o[:])

    nc.compile()
    return nc


def _get_program(key):
    prog = _prog_cache.get(key)
    if prog is None:
        prog = _build_program(key)
        _prog_cache[key] = prog
    return prog


_runner_cache: dict[tuple, dict] = {}
_dev_weights: dict[str, tuple] = {}
# inputs that rarely change between calls: keep them resident on-device
_WEIGHT_PARAMS = ("w23", "wp1")


def _get_runner(key, nc):
    """Compile-once executor for the SPMD program (the per-call jit rebuild in
    run_bass_kernel_spmd re-traces and re-compiles; this caches the compiled
    shard_map callable per bucket)."""
    r = _runner_cache.get(key)
    if r is not None:
        return r
    import jax
    import numpy as np
    from jax.sharding import Mesh, PartitionSpec
    from jax.experimental.shard_map import shard_map
    from concourse import bass2jax
    import concourse.mybir as mybir

    bass2jax.install_neuronx_cc_hook()
    partition_name = (nc.partition_id_tensor.name
                      if nc.partition_id_tensor else None)
    in_names, out_names, out_avals = [], [], []
    for alloc in nc.m.functions[0].allocations:
        if not isinstance(alloc, mybir.MemoryLocationSet):
            continue
        name = alloc.memorylocations[0].name
        if alloc.kind == "ExternalInput":
            if name != partition_name:
                in_names.append(name)
        elif alloc.kind == "ExternalOutput":
            out_names.append(name)
            out_avals.append(jax.core.ShapedArray(
                tuple(alloc.tensor_shape), mybir.dt.np(alloc.dtype)))
    n_params = len(in_names)
    all_names = in_names + out_names
    if partition_name is not None:
        all_names = all_names + [partition_name]
    all_names = tuple(all_names)

    def _body(*args):
        operands = list(args)
        if partition_name is not None:
            operands.append(bass2jax.partition_id_tensor())
        outs = bass2jax._bass_exec_p.bind(
            *operands, out_avals=tuple(out_avals), in_names=all_names,
            out_names=tuple(out_names), lowering_input_output_aliases=(),
            sim_require_finite=True, sim_require_nnan=True, nc=nc)
        return tuple(outs)

    devices = jax.devices()[:N_CORES]
    mesh = Mesh(np.asarray(devices), ("core",))
    in_specs = (PartitionSpec("core"),) * (n_params + len(out_names))
    out_specs = (PartitionSpec("core"),) * len(out_names)
    donate = tuple(range(n_params, n_params + len(out_names)))
    fn = jax.jit(
        shard_map(_body, mesh=mesh, in_specs=in_specs, out_specs=out_specs,
                  check_rep=False),
        donate_argnums=donate, keep_unused=True)
    r = dict(fn=fn, in_names=in_names, out_names=out_names,
             out_avals=out_avals, mesh=mesh)
    _runner_cache[key] = r
    return r


def _run_fast(key, nc, in_map):
    """Execute via the cached runner; returns core-0 output dict."""
    import jax
    from jax.sharding import NamedSharding, PartitionSpec

    r = _get_runner(key, nc)
    sharding = NamedSharding(r["mesh"], PartitionSpec("core"))
    args = []
    for name in r["in_names"]:
        arr = np.ascontiguousarray(in_map[name])
        if name in _WEIGHT_PARAMS:
            cached = _dev_weights.get(name)
            if cached is not None and cached[0].shape == arr.shape and \
                    np.array_equal(cached[0], arr):
                args.append(cached[1])
                continue
            dev = jax.device_put(
                np.concatenate([arr] * N_CORES, axis=0), sharding)
            _dev_weights[name] = (arr.copy(), dev)
            args.append(dev)
        else:
            args.append(np.concatenate([arr] * N_CORES, axis=0))
    zeros = [np.zeros((N_CORES * a.shape[0], *a.shape[1:]), a.dtype)
             for a in r["out_avals"]]
    outs = r["fn"](*args, *zeros)
    return {
        name: np.asarray(outs[i]).reshape(N_CORES, *r["out_avals"][i].shape)[0]
        for i, name in enumerate(r["out_names"])
    }


# --------------------------------------------------------------------------
# Host-side graph pruning / packing
# --------------------------------------------------------------------------

def _next_size(n, minimum):
    r = minimum
    while r < n:
        r *= 2
    return r


def _prune(N, src, dst, cvi):
    """Return (M, levels, edges, norms) for the 3-hop in-closure of M."""
    indeg = np.bincount(dst, minlength=N)
    deg = (1.0 + indeg).astype(np.float32)
    dinv = (1.0 / np.sqrt(deg)).astype(np.float32)
    self_norm = (1.0 / deg).astype(np.float32)

    M = np.unique(np.concatenate([dst[src == cvi], [cvi]]))

    order = np.argsort(dst, kind="stable")
    dst_sorted = dst[order]
    src_sorted = src[order]
    rowptr = np.zeros(N + 1, dtype=np.int64)
    np.cumsum(np.bincount(dst_sorted, minlength=N), out=rowptr[1:])

    def in_edges_of(nodes):
        cs, cd = [], []
        for n in nodes:
            s, e = rowptr[n], rowptr[n + 1]
            cs.append(src_sorted[s:e])
            cd.append(dst_sorted[s:e])
        if cs:
            return np.concatenate(cs), np.concatenate(cd)
        z = np.array([], np.int64)
        return z, z

    L3 = M
    e3s, e3d = in_edges_of(L3)
    L2 = np.unique(np.concatenate([L3, e3s]))
    e2s, e2d = in_edges_of(L2)
    L1 = np.unique(np.concatenate([L2, e2s]))
    e1s, e1d = in_edges_of(L1)
    L0 = np.unique(np.concatenate([L1, e1s]))

    return (M, (L0, L1, L2, L3),
            ((e1s, e1d), (e2s, e2d), (e3s, e3d)), (dinv, self_norm))


def _build_aggT(rows_nodes, cols_nodes, es, ed, dinv, self_norm, nr, ncol):
    """A.T zero-padded to [ncol, nr]: A[r,c] = sum(edge_norm) + self_norm diag."""
    AT = np.zeros((ncol, nr), np.float32)
    r = np.searchsorted(rows_nodes, ed)
    c = np.searchsorted(cols_nodes, es)
    w = dinv[es] * dinv[ed]
    np.add.at(AT, (c, r), w)
    rr = np.arange(len(rows_nodes))
    cc = np.searchsorted(cols_nodes, rows_nodes)
    AT[cc, rr] += self_norm[rows_nodes]
    return AT


def _tile128(a2d, k):
    """[k*128, f] -> [128, k*f] with tile j at columns [j*f, (j+1)*f)."""
    f = a2d.shape[1]
    return np.ascontiguousarray(
        a2d.reshape(k, 128, f).transpose(1, 0, 2).reshape(128, k * f))


def _numpy_fallback(vertices, src, dst, cvi, W1, b1, W2, b2, W3, b3,
                    Wp1, bp1, Wp2, bp2):
    """Identical-math pruned computation in numpy (used only for graphs whose
    closure exceeds the device bucket caps)."""
    N = vertices.shape[0]
    M, levels, edges, (dinv, self_norm) = _prune(N, src, dst, cvi)
    L0, L1, L2, L3 = levels

    def agg(h, rows, cols, es, ed):
        loc_c = np.searchsorted(cols, es)
        loc_r = np.searchsorted(rows, ed)
        out = np.zeros((len(rows), h.shape[1]), np.float32)
        np.add.at(out, loc_r, h[loc_c] * (dinv[es] * dinv[ed])[:, None])
        out += h[np.searchsorted(cols, rows)] * self_norm[rows][:, None]
        return out

    (e1s, e1d), (e2s, e2d), (e3s, e3d) = edges
    t0 = vertices[L0].astype(np.float32) @ W1
    h1 = np.tanh(agg(t0, L1, L0, e1s, e1d) + b1)
    t1 = h1 @ W2
    h2 = np.tanh(agg(t1, L2, L1, e2s, e2d) + b2)
    t2 = h2 @ W3
    h3 = np.tanh(agg(t2, L3, L2, e3s, e3d) + b3)
    p = h3 @ Wp1 + bp1
    p = np.where(p >= 0, p, ALPHA * p)
    s = (p @ Wp2 + bp2)[:, 0]
    s = s - s.max()
    e = np.exp(s)
    out = np.zeros(N, np.float32)
    out[M] = e / e.sum()
    return out


# --------------------------------------------------------------------------
# Entry point
# --------------------------------------------------------------------------

def kernel(**inputs) -> np.ndarray:
    global last_results
    vertices = np.ascontiguousarray(np.asarray(inputs["vertices"], np.float32))
    edge_index = np.asarray(inputs["edge_index"])
    cvi = int(np.asarray(inputs["current_vertex_idx"]))
    W1 = np.asarray(inputs["W1"], np.float32)
    W2 = np.asarray(inputs["W2"], np.float32)
    W3 = np.asarray(inputs["W3"], np.float32)
    Wp1 = np.asarray(inputs["Wp1"], np.float32)
    Wp2 = np.asarray(inputs["Wp2"], np.float32)
    b1 = np.asarray(inputs["b1"], np.float32)
    b2 = np.asarray(inputs["b2"], np.float32)
    b3 = np.asarray(inputs["b3"], np.float32)
    bp1 = np.asarray(inputs["bp1"], np.float32)
    bp2 = np.asarray(inputs["bp2"], np.float32)

    N = vertices.shape[0]
    src = np.asarray(edge_index[0], np.int64)
    dst = np.asarray(edge_index[1], np.int64)

    M, levels, edges, (dinv, self_norm) = _prune(N, src, dst, cvi)
    L0, L1, L2, L3 = levels
    (e1s, e1d), (e2s, e2d), (e3s, e3d) = edges

    n0 = _next_size(len(L0), 128)
    n1 = _next_size(len(L1), 64)
    n2 = _next_size(len(L2), 16)
    n3 = _next_size(len(L3), 8)
    # keep n1/n2 within the lo-half (<=64 partition rows)
    bucket = (n0, n1, n2, n3)
    lenM = len(M)
    if n0 > MAX_N0 or n1 > MAX_N1 or n2 > MAX_N2 or n3 > MAX_N3:
        return _numpy_fallback(vertices, src, dst, cvi, W1, b1, W2, b2,
                               W3, b3, Wp1, bp1, Wp2, bp2)
    k0 = n0 // 128
    zero_bias = bool(not b1.any() and not b2.any() and not b3.any()
                     and not bp1.any())
    key = (bucket, lenM, zero_bias)

    x0 = np.zeros((n0, F_IN), np.float32)
    x0[:len(L0)] = vertices[L0]
    a1t = _build_aggT(L1, L0, e1s, e1d, dinv, self_norm, n1, n0)
    a2t = _build_aggT(L2, L1, e2s, e2d, dinv, self_norm, n2, n1)
    a3t = _build_aggT(L3, L2, e3s, e3d, dinv, self_norm, n3, n2)

    lay = _blob_layout(bucket)
    blob = np.zeros((128, lay["_total"]), np.float32)
    blob[:, lay["x0"]:lay["x0"] + k0 * F_IN] = _tile128(x0, k0)
    blob[:, lay["a1"]:lay["a1"] + k0 * n1] = _tile128(a1t, k0)
    blob[:, lay["b1"]:lay["b1"] + KD] = b1.reshape(KD, 128).T
    blob[:n1, lay["a2"]:lay["a2"] + n2] = a2t
    blob[:, lay["b2"]:lay["b2"] + KD] = b2.reshape(KD, 128).T
    blob[:n2, lay["a3"]:lay["a3"] + n3] = a3t
    blob[:, lay["b3"]:lay["b3"] + KD] = b3.reshape(KD, 128).T
    blob[:, lay["bp1"]:lay["bp1"] + KH] = bp1.reshape(KH, 128).T
    blob[:, lay["wp2"]:lay["wp2"] + KH] = Wp2.reshape(KH, 128).T
    blob[:F_IN, lay["w1f"]:lay["w1f"] + D] = W1

    # wp1 packed for DoubleRow: [r, hi*2+p, g, c] = Wp1[(2p+g)*128 + r, hi*128+c]
    wp1r = np.empty((128, KH * 2, 2, 128), np.float32)
    for hi in range(KH):
        for p in range(KD // 2):
            for g in range(2):
                wp1r[:, hi * 2 + p, g, :] = \
                    Wp1[(2 * p + g) * 128:(2 * p + g + 1) * 128,
                        hi * 128:(hi + 1) * 128]

    w23 = np.empty((128, 2, KD, D), np.float32)
    w23[:, 0] = _tile128(W2, KD).reshape(128, KD, D)
    w23[:, 1] = _tile128(W3, KD).reshape(128, KD, D)

    blob16 = blob.astype(bf16)
    in_map = {
        "Alo": np.ascontiguousarray(blob16[0:64]),
        "Ahi": np.ascontiguousarray(blob16[64:128, :lay["_hi"]]),
        "w23": np.ascontiguousarray(w23.astype(fp8)),
        "wp1": np.ascontiguousarray(wp1r.astype(fp8)),
    }

    import os
    nc = _get_program(key)
    if os.environ.get("BASS_TRACE"):
        # profiling path (test harness): full run_bass_kernel_spmd with NTFF
        from concourse.bass_utils import run_bass_kernel_spmd
        last_results = run_bass_kernel_spmd(
            nc, [in_map] * N_CORES, list(range(N_CORES)))
        probs = np.asarray(last_results.results[0]["out"]).reshape(-1)
    else:
        out_map = _run_fast(key, nc, in_map)
        last_results = ("fast", out_map)
        probs = np.asarray(out_map["out"]).reshape(-1)

    out = np.zeros(N, np.float32)
    out[M] = probs[:lenM]
    return out


# revision 59
# speedup vs baseline: 1.0870x; 1.0599x over previous
"""Trainium2 Bass kernel for a 3-layer GCN + MLP scorer with neighbor-masked softmax.

The reference computes, for a graph with N nodes / E edges:
    h = tanh(GCN(tanh(GCN(tanh(GCN(x)))))); scores = MLP(h)
    out = softmax(scores masked to out-neighbors of current_vertex_idx)

The softmax mask makes the output exactly zero outside M = {out-neighbors of
cvi} | {cvi}.  Only the 3-hop *in*-neighborhood of M (a few hundred nodes of
the 50k) can influence the masked scores, so the kernel prunes the graph to
that closure on the host, builds small dense aggregation matrices (adjacency
with GCN normalization baked in), and runs the entire floating-point
computation on-device as a chain of dense matmuls + activations.  The device
program is SPMD-replicated across the 8 NeuronCores.

Host work is index-only (degree counts, BFS closure, packing the per-call
aggregation matrices); every FLOP of the model runs on the NeuronCores.

Device-side notes (v2):
  - All matmul operands are bf16 (fp32 PSUM accumulate): single-pass matmuls
    (fp32 needs 2 half-speed passes) and half the HBM traffic.  The softmax
    tail stays fp32.
  - Weights stream over both hardware DGE queues (sync + scalar), ordered by
    first use (W2 halves first, Wp1 last); W2/W3 are split into column halves
    so the first half of t = h @ W overlaps the second half's DMA.
  - Layer 1 is reassociated as (A1 @ x0) @ W1 (contract the node dim first at
    F_IN=16 wide); layouts alternate node-major/feature-major so every matmul
    contracts on partitions with no transposes.
  - Softmax is restricted to the first len(M) lanes (padding lanes are never
    read), which removes the -inf mask row; bp2 is dropped (softmax is
    invariant to constant shifts).  Leaky-relu is a single Prelu activation
    with the bias folded in via a K=1 matmul.
"""

import numpy as np
import ml_dtypes

D = 512      # node embedding size
H = 256      # predictor hidden size
F_IN = 16    # raw node feature dim
ALPHA = 0.1  # leaky relu slope
N_CORES = 8
KD = D // 128
KH = H // 128
N_WARMUP = 4  # dummy matmuls that ramp the PE clock during the DMA window

bf16 = ml_dtypes.bfloat16
fp8 = ml_dtypes.float8_e4m3

# Bucket caps: beyond these we fall back to the (identical-math) numpy path.
# n1/n2 <= 64 so the aggregation operands live in partition rows 0-63 (the
# "lo" half of the split input blob).
MAX_N0 = 4096
MAX_N1 = 64
MAX_N2 = 64
MAX_N3 = 64

_prog_cache: dict[tuple, object] = {}
last_results = None  # BassKernelResults of the most recent device run


def _blob_layout(bucket):
    """Column layout of the input blob A (bf16).  Columns [0, _hi) are
    needed on all 128 partition rows and are DMA'd as two row-halves (the
    DGE is descriptor-rate-bound, one descriptor per row, so row-splitting
    across both queues halves the wall time).  Columns [_hi, _total) only
    ever feed partition rows < 64 (w1 uses 16 rows, a2/a3 use n1/n2 <= 64)
    and ride only in the lo-half parameter."""
    n0, n1, n2, n3 = bucket
    k0 = n0 // 128
    off = 0
    lay = {}
    lay["x0"] = off; off += k0 * F_IN
    lay["a1"] = off; off += k0 * n1
    lay["b1"] = off; off += KD
    lay["b2"] = off; off += KD
    lay["b3"] = off; off += KD
    lay["bp1"] = off; off += KH     # bp1 feature-major columns
    lay["wp2"] = off; off += KH     # feature-major columns
    lay["_hi"] = off                # hi-half parameter covers [0, _hi)
    lay["w1f"] = off; off += D      # W1 [16, 512] in partition rows 0-15
    lay["a2"] = off; off += n2      # rows 0..n1
    lay["a3"] = off; off += n3      # rows 0..n2
    lay["_total"] = off
    return lay


# --------------------------------------------------------------------------
# Device program
# --------------------------------------------------------------------------

def _build_program(key):
    import concourse.bass as bass
    import concourse.bacc as bacc
    import concourse.mybir as mybir
    import concourse.tile as tile

    bucket, lenM, zero_bias = key
    n0, n1, n2, n3 = bucket
    f32 = mybir.dt.float32
    b16 = mybir.dt.bfloat16
    f8 = mybir.dt.float8e4
    k0 = n0 // 128
    Tanh = mybir.ActivationFunctionType.Tanh
    Prelu = mybir.ActivationFunctionType.Prelu
    Exp = mybir.ActivationFunctionType.Exp
    Copy = mybir.ActivationFunctionType.Copy
    DR = mybir.MatmulPerfMode.DoubleRow
    lay = _blob_layout(bucket)
    CA = lay["_total"]
    CH = lay["_hi"]
    DH = D // 2   # 256: t1/t2 PSUM->SBUF cast half width

    nc = bacc.Bacc("TRN2", target_bir_lowering=False, debug=False)
    P_Alo = nc.declare_dram_parameter("Alo", [64, CA], b16, isOutput=False)
    P_Ahi = nc.declare_dram_parameter("Ahi", [64, CH], b16, isOutput=False)
    P_w23 = nc.declare_dram_parameter("w23", [128, 2, KD, D], f8,
                                      isOutput=False)
    P_wp1 = nc.declare_dram_parameter("wp1", [128, KH * 2, 2, 128], f8,
                                      isOutput=False)
    P_out = nc.declare_dram_parameter("out", [1, lenM], f32, isOutput=True)

    with tile.TileContext(nc) as tc:
        with (
            tc.tile_pool(name="sb", bufs=1) as sb,
            tc.tile_pool(name="ps", bufs=4, space="PSUM") as ps,
            tc.tile_pool(name="pss", bufs=2, space="PSUM") as pss,
            tc.tile_pool(name="wu", bufs=1, space="PSUM") as wu,
        ):
            # ---- input DMAs: the DGE is descriptor-rate-bound (one
            # descriptor per partition row), so the blob A (whose delivery
            # gates the whole H1 chain) is row-split across both queues;
            # W2|W3 (4KB descriptors) fill the sync queue's second slot and
            # wp1 the scalar queue's.
            A = sb.tile([128, CA], b16, tag="A")
            nc.sync.dma_start(A[0:64, :], P_Alo[:])
            nc.scalar.dma_start(A[64:128, 0:CH], P_Ahi[:])
            w23 = sb.tile([128, 2, KD, D], f8, tag="w23")
            nc.sync.dma_start(w23[:], P_w23[:])
            wp1 = sb.tile([128, KH * 2, 2, 128], f8, tag="wp1")
            nc.scalar.dma_start(wp1[:], P_wp1[:])

            # ---- PE warm-up: the tensor engine's clock ramps with use (low ->
            # mid p-state); a fat dummy absorbs the ramp and short keep-alives
            # hold the clock until the first real matmul.
            wu_sb = sb.tile([128, 512], b16, tag="wu_sb")
            nc.vector.memset(wu_sb[:], 0)
            wu_ps = wu.tile([128, 512], f32, tag="wu_ps")
            for i in range(N_WARMUP):
                w_cols = 512 if i < 1 else 64
                nc.tensor.matmul(wu_ps[:, 0:w_cols], wu_sb[:, 0:128],
                                 wu_sb[:, 0:w_cols], start=True, stop=True)
            wu_out = sb.tile([1, 1], f32, tag="wu_out")
            nc.vector.tensor_copy(wu_out[:], wu_ps[0:1, 0:1])

            def acol(name, i=0, w=1, p0=0, p1=128):
                return A[p0:p1, lay[name] + i:lay[name] + i + w]

            def act_bias(out_ap, acc_ap, func, bname, di, **kw):
                if zero_bias:
                    nc.scalar.activation(out_ap, acc_ap, func, **kw)
                else:
                    nc.scalar.activation(out_ap, acc_ap, func,
                                         bias=acol(bname, di), **kw)

            # ---- BT = (A1 @ x0).T : [F_IN, n1]  (contract n0 nodes)
            acc = pss.tile([F_IN, n1], f32, tag="pss")
            for j in range(k0):
                nc.tensor.matmul(acc[:], acol("x0", j * F_IN, F_IN),
                                 acol("a1", j * n1, n1),
                                 start=(j == 0), stop=(j == k0 - 1))
            bt = sb.tile([F_IN, n1], b16, tag="bt")
            nc.vector.tensor_copy(bt[:], acc[:])

            # ---- H1[:, di] = tanh(W1[:, dslice].T @ BT + b1) : feature-major
            # fp8 so the dense layers can run DoubleRow (2 K-rows/cycle).
            # Tanhs run per d-tile PAIR (separate PSUM tiles per pair: a
            # group's start=True zeroes bank-granular regions, so mixing
            # groups and readers in one bank serializes them).
            H1 = sb.tile([128, KD, n1], f8, tag="H1")
            for pr in range(KD // 2):
                accp2 = ps.tile([128, 2, n1], f32, tag="ps")
                for g in range(2):
                    di = 2 * pr + g
                    nc.tensor.matmul(accp2[:, g, :],
                                     acol("w1f", di * 128, 128, 0, F_IN),
                                     bt[:], start=True, stop=True)
                    if not zero_bias:
                        act_bias(H1[:, di, :], accp2[:, g, :], Tanh, "b1", di)
                if zero_bias:
                    nc.scalar.activation(H1[:, 2 * pr:2 * pr + 2, :],
                                         accp2[:], Tanh)

            def dense_layer(Hf, n_rows, wi, lname):
                # t = h @ W : node-major [n_rows, D] via fp8 DoubleRow pairs;
                # PSUM->SBUF bf16 cast halves on vector+scalar engines.
                t_sb = sb.tile([n_rows, D], b16, tag=lname)
                acc = ps.tile([n_rows, D], f32, tag="ps")
                for p in range(KD // 2):
                    nc.tensor.matmul(acc[:], Hf[:, 2 * p:2 * p + 2, :],
                                     w23[:, wi, 2 * p:2 * p + 2, :],
                                     start=(p == 0), stop=(p == KD // 2 - 1),
                                     perf_mode=DR)
                nc.vector.tensor_copy(t_sb[:, 0:DH], acc[:, 0:DH])
                nc.vector.tensor_copy(t_sb[:, DH:D], acc[:, DH:D])
                return t_sb

            def agg_layer(t_in, n_in, aname, n_out, bname, lname):
                # Hf[:, di] = tanh((A @ t).T + b) : feature-major, fp8,
                # per-pair PSUM tiles + tanh so t(next) pair 0 starts early
                Hf = sb.tile([128, KD, n_out], f8, tag=lname)
                for pr in range(KD // 2):
                    accp2 = ps.tile([128, 2, n_out], f32, tag="ps")
                    for g in range(2):
                        di = 2 * pr + g
                        nc.tensor.matmul(accp2[:, g, :],
                                         t_in[:, di * 128:(di + 1) * 128],
                                         acol(aname, 0, n_out, 0, n_in),
                                         start=True, stop=True)
                        if not zero_bias:
                            act_bias(Hf[:, di, :], accp2[:, g, :], Tanh,
                                     bname, di)
                    if zero_bias:
                        nc.scalar.activation(Hf[:, 2 * pr:2 * pr + 2, :],
                                             accp2[:], Tanh)
                return Hf

            t1 = dense_layer(H1, n1, 0, "t1")
            H2 = agg_layer(t1, n1, "a2", n2, "b2", "H2")
            t2 = dense_layer(H2, n2, 1, "t2")
            H3 = agg_layer(t2, n2, "a3", n3, "b3", "H3")

            # ---- predictor hidden: P[:, hi] = prelu(Wp1.T @ h3.T + bp1)
            # (fp8 DoubleRow over d-tile pairs; bp1 rides the activation bias)
            Pf = sb.tile([128, KH * n3], b16, tag="Pf")
            for hi in range(KH):
                accp = ps.tile([128, n3], f32, tag="ps")
                for p in range(KD // 2):
                    nc.tensor.matmul(
                        accp[:], wp1[:, hi * 2 + p, :, :],
                        H3[:, 2 * p:2 * p + 2, :], start=(p == 0),
                        stop=(p == KD // 2 - 1), perf_mode=DR)
                act_bias(Pf[:, hi * n3:(hi + 1) * n3], accp[:],
                         Prelu, "bp1", hi, alpha=ALPHA)

            # ---- scores (first lenM lanes only; softmax is shift-invariant
            # so bp2 is dropped, and padding lanes are never read)
            acc = pss.tile([1, n3], f32, tag="pss")
            for hi in range(KH):
                nc.tensor.matmul(acc[:], acol("wp2", hi),
                                 Pf[:, hi * n3:(hi + 1) * n3],
                                 start=(hi == 0), stop=(hi == KH - 1))
            e = sb.tile([1, lenM], f32, tag="e")
            nc.scalar.activation(e[:], acc[0:1, 0:lenM], Exp)
            ssum = sb.tile([1, 1], f32, tag="ssum")
            nc.vector.tensor_reduce(ssum[:], e[:], mybir.AxisListType.X,
                                    mybir.AluOpType.add)
            rs = sb.tile([1, 1], f32, tag="rs")
            nc.vector.reciprocal(rs[:], ssum[:])
            o = sb.tile([1, lenM], f32, tag="o")
            nc.vector.tensor_scalar_mul(o[:], e[:], rs[:])
            nc.sync.dma_start(P_out[:], o[:])

    nc.compile()
    return nc


def _get_program(key):
    prog = _prog_cache.get(key)
    if prog is None:
        prog = _build_program(key)
        _prog_cache[key] = prog
    return prog


_runner_cache: dict[tuple, dict] = {}
_dev_weights: dict[str, tuple] = {}
# inputs that rarely change between calls: keep them resident on-device
_WEIGHT_PARAMS = ("w23", "wp1")


def _get_runner(key, nc):
    """Compile-once executor for the SPMD program (the per-call jit rebuild in
    run_bass_kernel_spmd re-traces and re-compiles; this caches the compiled
    shard_map callable per bucket)."""
    r = _runner_cache.get(key)
    if r is not None:
        return r
    import jax
    import numpy as np
    from jax.sharding import Mesh, PartitionSpec
    from jax.experimental.shard_map import shard_map
    from concourse import bass2jax
    import concourse.mybir as mybir

    bass2jax.install_neuronx_cc_hook()
    partition_name = (nc.partition_id_tensor.name
                      if nc.partition_id_tensor else None)
    in_names, out_names, out_avals = [], [], []
    for alloc in nc.m.functions[0].allocations:
        if not isinstance(alloc, mybir.MemoryLocationSet):
            continue
        name = alloc.memorylocations[0].name
        if alloc.kind == "ExternalInput":
            if name != partition_name:
                in_names.append(name)
        elif alloc.kind == "ExternalOutput":
            out_names.append(name)
            out_avals.append(jax.core.ShapedArray(
                tuple(alloc.tensor_shape), mybir.dt.np(alloc.dtype)))
    n_params = len(in_names)
    all_names = in_names + out_names
    if partition_name is not None:
        all_names = all_names + [partition_name]
    all_names = tuple(all_names)

    def _body(*args):
        operands = list(args)
        if partition_name is not None:
            operands.append(bass2jax.partition_id_tensor())
        outs = bass2jax._bass_exec_p.bind(
            *operands, out_avals=tuple(out_avals), in_names=all_names,
            out_names=tuple(out_names), lowering_input_output_aliases=(),
            sim_require_finite=True, sim_require_nnan=True, nc=nc)
        return tuple(outs)

    devices = jax.devices()[:N_CORES]
    mesh = Mesh(np.asarray(devices), ("core",))
    in_specs = (PartitionSpec("core"),) * (n_params + len(out_names))
    out_specs = (PartitionSpec("core"),) * len(out_names)
    donate = tuple(range(n_params, n_params + len(out_names)))
    fn = jax.jit(
        shard_map(_body, mesh=mesh, in_specs=in_specs, out_specs=out_specs,
                  check_rep=False),
        donate_argnums=donate, keep_unused=True)
    r = dict(fn=fn, in_names=in_names, out_names=out_names,
             out_avals=out_avals, mesh=mesh)
    _runner_cache[key] = r
    return r


def _run_fast(key, nc, in_map):
    """Execute via the cached runner; returns core-0 output dict."""
    import jax
    from jax.sharding import NamedSharding, PartitionSpec

    r = _get_runner(key, nc)
    sharding = NamedSharding(r["mesh"], PartitionSpec("core"))
    args = []
    for name in r["in_names"]:
        arr = np.ascontiguousarray(in_map[name])
        if name in _WEIGHT_PARAMS:
            cached = _dev_weights.get(name)
            if cached is not None and cached[0].shape == arr.shape and \
                    np.array_equal(cached[0], arr):
                args.append(cached[1])
                continue
            dev = jax.device_put(
                np.concatenate([arr] * N_CORES, axis=0), sharding)
            _dev_weights[name] = (arr.copy(), dev)
            args.append(dev)
        else:
            args.append(np.concatenate([arr] * N_CORES, axis=0))
    zeros = [np.zeros((N_CORES * a.shape[0], *a.shape[1:]), a.dtype)
             for a in r["out_avals"]]
    outs = r["fn"](*args, *zeros)
    return {
        name: np.asarray(outs[i]).reshape(N_CORES, *r["out_avals"][i].shape)[0]
        for i, name in enumerate(r["out_names"])
    }


# --------------------------------------------------------------------------
# Host-side graph pruning / packing
# --------------------------------------------------------------------------

def _next_size(n, minimum):
    r = minimum
    while r < n:
        r *= 2
    return r


def _prune(N, src, dst, cvi):
    """Return (M, levels, edges, norms) for the 3-hop in-closure of M."""
    indeg = np.bincount(dst, minlength=N)
    deg = (1.0 + indeg).astype(np.float32)
    dinv = (1.0 / np.sqrt(deg)).astype(np.float32)
    self_norm = (1.0 / deg).astype(np.float32)

    M = np.unique(np.concatenate([dst[src == cvi], [cvi]]))

    order = np.argsort(dst, kind="stable")
    dst_sorted = dst[order]
    src_sorted = src[order]
    rowptr = np.zeros(N + 1, dtype=np.int64)
    np.cumsum(np.bincount(dst_sorted, minlength=N), out=rowptr[1:])

    def in_edges_of(nodes):
        cs, cd = [], []
        for n in nodes:
            s, e = rowptr[n], rowptr[n + 1]
            cs.append(src_sorted[s:e])
            cd.append(dst_sorted[s:e])
        if cs:
            return np.concatenate(cs), np.concatenate(cd)
        z = np.array([], np.int64)
        return z, z

    L3 = M
    e3s, e3d = in_edges_of(L3)
    L2 = np.unique(np.concatenate([L3, e3s]))
    e2s, e2d = in_edges_of(L2)
    L1 = np.unique(np.concatenate([L2, e2s]))
    e1s, e1d = in_edges_of(L1)
    L0 = np.unique(np.concatenate([L1, e1s]))

    return (M, (L0, L1, L2, L3),
            ((e1s, e1d), (e2s, e2d), (e3s, e3d)), (dinv, self_norm))


def _build_aggT(rows_nodes, cols_nodes, es, ed, dinv, self_norm, nr, ncol):
    """A.T zero-padded to [ncol, nr]: A[r,c] = sum(edge_norm) + self_norm diag."""
    AT = np.zeros((ncol, nr), np.float32)
    r = np.searchsorted(rows_nodes, ed)
    c = np.searchsorted(cols_nodes, es)
    w = dinv[es] * dinv[ed]
    np.add.at(AT, (c, r), w)
    rr = np.arange(len(rows_nodes))
    cc = np.searchsorted(cols_nodes, rows_nodes)
    AT[cc, rr] += self_norm[rows_nodes]
    return AT


def _tile128(a2d, k):
    """[k*128, f] -> [128, k*f] with tile j at columns [j*f, (j+1)*f)."""
    f = a2d.shape[1]
    return np.ascontiguousarray(
        a2d.reshape(k, 128, f).transpose(1, 0, 2).reshape(128, k * f))


def _numpy_fallback(vertices, src, dst, cvi, W1, b1, W2, b2, W3, b3,
                    Wp1, bp1, Wp2, bp2):
    """Identical-math pruned computation in numpy (used only for graphs whose
    closure exceeds the device bucket caps)."""
    N = vertices.shape[0]
    M, levels, edges, (dinv, self_norm) = _prune(N, src, dst, cvi)
    L0, L1, L2, L3 = levels

    def agg(h, rows, cols, es, ed):
        loc_c = np.searchsorted(cols, es)
        loc_r = np.searchsorted(rows, ed)
        out = np.zeros((len(rows), h.shape[1]), np.float32)
        np.add.at(out, loc_r, h[loc_c] * (dinv[es] * dinv[ed])[:, None])
        out += h[np.searchsorted(cols, rows)] * self_norm[rows][:, None]
        return out

    (e1s, e1d), (e2s, e2d), (e3s, e3d) = edges
    t0 = vertices[L0].astype(np.float32) @ W1
    h1 = np.tanh(agg(t0, L1, L0, e1s, e1d) + b1)
    t1 = h1 @ W2
    h2 = np.tanh(agg(t1, L2, L1, e2s, e2d) + b2)
    t2 = h2 @ W3
    h3 = np.tanh(agg(t2, L3, L2, e3s, e3d) + b3)
    p = h3 @ Wp1 + bp1
    p = np.where(p >= 0, p, ALPHA * p)
    s = (p @ Wp2 + bp2)[:, 0]
    s = s - s.max()
    e = np.exp(s)
    out = np.zeros(N, np.float32)
    out[M] = e / e.sum()
    return out


# --------------------------------------------------------------------------
# Entry point
# --------------------------------------------------------------------------

def kernel(**inputs) -> np.ndarray:
    global last_results
    vertices = np.ascontiguousarray(np.asarray(inputs["vertices"], np.float32))
    edge_index = np.asarray(inputs["edge_index"])
    cvi = int(np.asarray(inputs["current_vertex_idx"]))
    W1 = np.asarray(inputs["W1"], np.float32)
    W2 = np.asarray(inputs["W2"], np.float32)
    W3 = np.asarray(inputs["W3"], np.float32)
    Wp1 = np.asarray(inputs["Wp1"], np.float32)
    Wp2 = np.asarray(inputs["Wp2"], np.float32)
    b1 = np.asarray(inputs["b1"], np.float32)
    b2 = np.asarray(inputs["b2"], np.float32)
    b3 = np.asarray(inputs["b3"], np.float32)
    bp1 = np.asarray(inputs["bp1"], np.float32)
    bp2 = np.asarray(inputs["bp2"], np.float32)

    N = vertices.shape[0]
    src = np.asarray(edge_index[0], np.int64)
    dst = np.asarray(edge_index[1], np.int64)

    M, levels, edges, (dinv, self_norm) = _prune(N, src, dst, cvi)
    L0, L1, L2, L3 = levels
    (e1s, e1d), (e2s, e2d), (e3s, e3d) = edges

    n0 = _next_size(len(L0), 128)
    n1 = _next_size(len(L1), 64)
    n2 = _next_size(len(L2), 16)
    n3 = _next_size(len(L3), 8)
    # keep n1/n2 within the lo-half (<=64 partition rows)
    bucket = (n0, n1, n2, n3)
    lenM = len(M)
    if n0 > MAX_N0 or n1 > MAX_N1 or n2 > MAX_N2 or n3 > MAX_N3:
        return _numpy_fallback(vertices, src, dst, cvi, W1, b1, W2, b2,
                               W3, b3, Wp1, bp1, Wp2, bp2)
    k0 = n0 // 128
    zero_bias = bool(not b1.any() and not b2.any() and not b3.any()
                     and not bp1.any())
    key = (bucket, lenM, zero_bias)

    x0 = np.zeros((n0, F_IN), np.float32)
    x0[:len(L0)] = vertices[L0]
    a1t = _build_aggT(L1, L0, e1s, e1d, dinv, self_norm, n1, n0)
    a2t = _build_aggT(L2, L1, e2s, e2d, dinv, self_norm, n2, n1)
    a3t = _build_aggT(L3, L2, e3s, e3d, dinv, self_norm, n3, n2)

    lay = _blob_layout(bucket)
    blob = np.zeros((128, lay["_total"]), np.float32)
    blob[:, lay["x0"]:lay["x0"] + k0 * F_IN] = _tile128(x0, k0)
    blob[:, lay["a1"]:lay["a1"] + k0 * n1] = _tile128(a1t, k0)
    blob[:, lay["b1"]:lay["b1"] + KD] = b1.reshape(KD, 128).T
    blob[:n1, lay["a2"]:lay["a2"] + n2] = a2t
    blob[:, lay["b2"]:lay["b2"] + KD] = b2.reshape(KD, 128).T
    blob[:n2, lay["a3"]:lay["a3"] + n3] = a3t
    blob[:, lay["b3"]:lay["b3"] + KD] = b3.reshape(KD, 128).T
    blob[:, lay["bp1"]:lay["bp1"] + KH] = bp1.reshape(KH, 128).T
    blob[:, lay["wp2"]:lay["wp2"] + KH] = Wp2.reshape(KH, 128).T
    blob[:F_IN, lay["w1f"]:lay["w1f"] + D] = W1

    # wp1 packed for DoubleRow: [r, hi*2+p, g, c] = Wp1[(2p+g)*128 + r, hi*128+c]
    wp1r = np.empty((128, KH * 2, 2, 128), np.float32)
    for hi in range(KH):
        for p in range(KD // 2):
            for g in range(2):
                wp1r[:, hi * 2 + p, g, :] = \
                    Wp1[(2 * p + g) * 128:(2 * p + g + 1) * 128,
                        hi * 128:(hi + 1) * 128]

    w23 = np.empty((128, 2, KD, D), np.float32)
    w23[:, 0] = _tile128(W2, KD).reshape(128, KD, D)
    w23[:, 1] = _tile128(W3, KD).reshape(128, KD, D)

    blob16 = blob.astype(bf16)
    in_map = {
        "Alo": np.ascontiguousarray(blob16[0:64]),
        "Ahi": np.ascontiguousarray(blob16[64:128, :lay["_hi"]]),
        "w23": np.ascontiguousarray(w23.astype(fp8)),
        "wp1": np.ascontiguousarray(wp1r.astype(fp8)),
    }

    import os
    nc = _get_program(key)
    if os.environ.get("BASS_TRACE"):
        # profiling path (test harness): full run_bass_kernel_spmd with NTFF
        from concourse.bass_utils import run_bass_kernel_spmd
        last_results = run_bass_kernel_spmd(
            nc, [in_map] * N_CORES, list(range(N_CORES)))
        probs = np.asarray(last_results.results[0]["out"]).reshape(-1)
    else:
        out_map = _run_fast(key, nc, in_map)
        last_results = ("fast", out_map)
        probs = np.asarray(out_map["out"]).reshape(-1)

    out = np.zeros(N, np.float32)
    out[M] = probs[:lenM]
    return out
